# revision 1
# baseline (speedup 1.0000x reference)
"""Trainium2 Bass kernel v2 for nn_CrispToFuzzyConv (hypergraph message passing).

Math (see reference):
  Xe   = segment_sum(X[vertex], edges, E)                 # round 1
  Xv   = concat([deg * X, Xv2]),  Xv2 = segment_sum(Xe[edges], vertex, N)
  center = Xv @ w_b + b_b
  HL = center - (|Xv| @ w_a + b_a)
  HR = center + (|Xv| @ w_c + b_c)

v2 strategy (vs v1 which used dma_scatter_add):
  - NO scatter-adds.  Segment sums are computed as one-hot matmuls on the
    PE: gathered token rows [128tok x F] are contracted against on-chip
    one-hot matrices.  All of one gather call's one-hot blocks are built
    with a SINGLE DVE tensor_tensor:
        oh[p, (e, j)] = (iota_rep[j] == segmb[p, e])     # stride-0 bcast
    where segmb = segid - (window*512 + sub*128) is precomputed host-side
    per mm entry (DVE instruction overhead ~0.5-1us dwarfs the per-element
    cost, so batching 5-15 builds into one op is ~10x cheaper).
  - For stage A the PSUM window is Xe^T [F x 512 edge-cols]; for stage C
    it is Xv2^T [F x 512 node-cols] which feeds the dense head directly
    as matmul lhsT (no transposes in the dense path; the host supplies
    deg-scaled X^T).
  - Everything is bf16 on the wire (gather rows are 256B packets, the
    AllGather payload halves); PSUM accumulates in f32.
  - SPMD: one program for all 8 cores.  The schedule is static, sized by
    the max token count over cores per (window, chunk/region) cell; each
    cell is one 1024-slot gather call whose runtime count (num_idxs_reg)
    is that max, so padding costs ~3% packets, not 30%.
  - Stage C accumulates region passes into an SBUF-resident Xv2 table
    (24.5KB/partition) so PSUM windows stay short-lived; the dense head
    runs fused per 128-node group right after its window closes.
  - Two AllGathers (edge regions) pipeline behind stage A / stage C
    descriptor generation.

Hardware constraints baked in (v1 + this session's trace/sim analysis):
  - dma_gather idx dtype is int16 -> gather tables <= 32767 rows: X is
    split in 4 chunks of 25000, Xe tables in 2 regions (24576/25600 rows)
  - elem_size_bytes % 256 == 0 -> bf16 F=128 rows (256B) are legal
  - gather output layout: token i -> partition i%128, col-block i//128,
    which is exactly the PE contraction layout
  - SWDGE requires num_idxs_reg == #(idx >= 0) and valid idxs contiguous
    from slot 0 (trailing -1 pads); slots past the count are never
    written, so the boundary block's tail is zero-filled via a small DMA
    (engine ops can't start at arbitrary partitions)
  - PSUM accumulation "zero regions" are 2KB (a whole bank): start/stop
    flags bracket a window's full mm list, and every PSUM tile is padded
    to a full bank so start_tensor_calc can't corrupt a neighbor
  - the Tile sem-assignment pass locks each DMASW semaphore lane to one
    SWDGE queue; lanes rotate over Pool-engine DMA insts in SCHEDULED
    order, so queue_num is assigned AFTER scheduling as lane % 4
  - per-instruction overhead (~0.3-1us) dominates small ops: one-hot
    builds are batched per call, DMA triggers are spread across sync /
    scalar engines
"""

import numpy as np

# ---------------------------------------------------------------- constants
N = 100000
E = 50000
NNZ = 300000
F = 128
NC = 8

NODE_SH = 12500              # nodes per core
NODE_SH_P = 12544            # 98 groups of 128
NGRP = 98
CW = 25                      # 512-node windows (last = 256)

EDGE_SH = 6250               # edges per core
EDGE_SH_P = 6272             # 49 subwindows of 128
AW = 13                      # 512-edge windows (last = 128)
NCH = 4
CHUNK = 25000                # X chunk rows (int16 gather limit)

REG_SPLIT = 3072             # local edge rows [0,3072) -> region 0
REG_ROWS = (3072, 3200)      # padded local rows per region
TBL_ROWS = (NC * 3072, NC * 3200)

T = 1024                     # slots per gather call (8 blocks of 128)

_STATE = {}


# ---------------------------------------------------------------- host side
def _wrap16(idx):
    """[ncall, 1024] int -> [128, ncall*64] int16 SBUF image (idx i at
    partition i%16, col i//16; replicated across the 8 groups of 16)."""
    ncall = idx.shape[0]
    t = idx.reshape(ncall, 64, 16).transpose(0, 2, 1).astype(np.int16)
    t = np.tile(t, (1, 8, 1))                      # [ncall, 128, 64]
    return np.ascontiguousarray(t.transpose(1, 0, 2).reshape(128, ncall * 64))


def _segmb_img(sg, entries, emax):
    """Per-entry biased segid image: [128, ncall*emax] f32 where column
    k*emax+e holds segid(block(e)) - base(e) for call k (never a valid
    one-hot match for pad slots: segid -1 -> negative)."""
    ncall = sg.shape[0]
    img = np.full((128, ncall * emax), -3.0e6, np.float32)
    for k in range(ncall):
        for e, (b, base) in enumerate(entries[k]):
            img[:, k * emax + e] = sg[k, b * 128:(b + 1) * 128] - base
    return np.ascontiguousarray(img)


def _build_stream(owner, cell, gidx, segid, ncells):
    """Pack tokens into per-(core, cell) sorted slot arrays."""
    ga = np.zeros((NC, ncells, T), np.int64)
    sg = np.full((NC, ncells, T), -1.0, np.float32)
    cnt = np.zeros((NC, ncells), np.int64)
    for m in range(NC):
        s = np.nonzero(owner == m)[0]
        o = np.lexsort((segid[s], cell[s]))
        s = s[o]
        cs = cell[s]
        bounds = np.searchsorted(cs, np.arange(ncells + 1))
        for k in range(ncells):
            lo, hi = int(bounds[k]), int(bounds[k + 1])
            n = hi - lo
            if n > T:
                return None
            ga[m, k, :n] = gidx[s[lo:hi]]
            sg[m, k, :n] = segid[s[lo:hi]]
            cnt[m, k] = n
    return ga, sg, cnt


def _cell_entries(sg_all, k, cnt_max, wbase, nsub):
    """Static (block, sub) entry list for one cell, from the union of all
    cores' segids.  Returns None if a segid falls outside the window."""
    entries = []
    nb = -(-int(cnt_max) // 128)
    for b in range(nb):
        vals = sg_all[:, k, b * 128:(b + 1) * 128].ravel()
        vals = vals[vals >= 0].astype(np.int64)
        if len(vals) == 0:
            continue
        for sub in np.unique(vals // 128):
            s = int(sub) - wbase // 128
            if s < 0 or s >= nsub:
                return None
            entries.append((b, s))
    return entries


def _route(vertex, edges):
    eo = edges // EDGE_SH
    le = edges % EDGE_SH
    vo = vertex // NODE_SH
    lv = vertex % NODE_SH
    ch = vertex // CHUNK
    reg = (le >= REG_SPLIT).astype(np.int64)
    trow = np.where(reg == 0, eo * REG_ROWS[0] + le,
                    eo * REG_ROWS[1] + (le - REG_SPLIT))

    rA = _build_stream(eo, (le // 512) * 4 + ch, vertex - ch * CHUNK,
                       le.astype(np.float32), AW * NCH)
    if rA is None:
        return None
    gaA, sgA, cntA = rA
    rC = _build_stream(vo, reg * CW + lv // 512, trow,
                       lv.astype(np.float32), 2 * CW)
    if rC is None:
        return None
    gaC, sgC, cntC = rC

    # Round static counts up to a multiple of 128 (idx-0 pad slots, segid
    # -1): every mm-read block is then fully DMA-written, so no stale-slot
    # zero-fill (and its cross-engine dep hop) is needed.
    cntA_max = np.minimum(-(-cntA.max(axis=0) // 128) * 128, T)
    cntC_max = np.minimum(-(-cntC.max(axis=0) // 128) * 128, T)
    # Slots beyond the static (max-over-cores) count must be NEGATIVE: the
    # SWDGE ucode requires num_idxs_reg == #(idx >= 0) and stops at the
    # last non-negative slot.  Slots in [cnt_m, cnt_max) keep idx 0 (valid
    # row; their segid is -1 so the one-hot zeroes them out).
    for k in range(AW * NCH):
        gaA[:, k, int(cntA_max[k]):] = -1
    for k in range(2 * CW):
        gaC[:, k, int(cntC_max[k]):] = -1

    def merge_runs(ents):
        # consecutive entries of the same block with consecutive subs are
        # one matmul (contiguous oh slice -> contiguous psum cols)
        runs = []
        i = 0
        while i < len(ents):
            b, s = ents[i]
            n = 1
            while i + n < len(ents) and ents[i + n] == (b, s + n):
                n += 1
            runs.append((i, b, s, n))
            i += n
        return runs

    # Entry lists per call: (block, sub) plus base for segmb biasing.
    entA = []          # per call: [(b, base)]
    mmA = []           # per window: [(cell, e0, block, s0, nsp, st, sp)]
    for w in range(AW):
        nsub = 4 if w < AW - 1 else 1
        wlist = []
        subs_seen = set()
        for c in range(NCH):
            k = w * 4 + c
            ents = _cell_entries(sgA, k, cntA_max[k], w * 512, nsub)
            if ents is None:
                return None
            entA.append([(b, w * 512 + s * 128) for (b, s) in ents])
            subs_seen |= set(s for (_, s) in ents)
            wlist += [(k, e0, b, s0, n) for (e0, b, s0, n) in merge_runs(ents)]
        if subs_seen != set(range(nsub)):
            return None
        mmA.append([(k, e0, b, s0, n, i == 0, i == len(wlist) - 1)
                    for i, (k, e0, b, s0, n) in enumerate(wlist)])

    entC = []
    mmC = []           # per cell k: [(e0, block, s0, nsp, st, sp)]
    for r in range(2):
        for w in range(CW):
            nsub = 4 if w < CW - 1 else 2
            k = r * CW + w
            ents = _cell_entries(sgC, k, cntC_max[k], w * 512, nsub)
            if ents is None or not ents:
                return None
            if set(s for (_, s) in ents) != set(range(nsub)):
                return None
            entC.append([(b, w * 512 + s * 128) for (b, s) in ents])
            runs = merge_runs(ents)
            mmC.append([(e0, b, s0, n, i == 0, i == len(runs) - 1)
                        for i, (e0, b, s0, n) in enumerate(runs)])

    emaxA = max(len(x) for x in entA)
    emaxC = max(len(x) for x in entC)
    sig = repr((tuple(cntA_max), tuple(cntC_max), mmA, mmC, emaxA, emaxC))
    return dict(gaA=gaA, sgA=sgA, cntA=cntA_max, mmA=mmA, entA=entA,
                gaC=gaC, sgC=sgC, cntC=cntC_max, mmC=mmC, entC=entC,
                emaxA=emaxA, emaxC=emaxC, sig=sig)


def _numpy_fallback(X, vertex, edges, w_b, w_a, w_c, b_b, b_a, b_c):
    Xe = np.zeros((E, F), np.float32)
    np.add.at(Xe, edges, X[vertex])
    Xv2 = np.zeros((N, F), np.float32)
    np.add.at(Xv2, vertex, Xe[edges])
    deg = np.bincount(vertex, minlength=N).astype(np.float32)[:, None]
    Xv = np.concatenate([deg * X, Xv2], axis=1)
    center = Xv @ w_b + b_b
    aXv = np.abs(Xv)
    return (center.astype(np.float32),
            (center - (aXv @ w_a + b_a)).astype(np.float32),
            (center + (aXv @ w_c + b_c)).astype(np.float32))


# ------------------------------------------------------------- bass program
def _build_program(meta):
    from concourse import bacc, tile
    import concourse.mybir as mybir

    f32 = mybir.dt.float32
    bf16 = mybir.dt.bfloat16
    i16 = mybir.dt.int16
    Alu = mybir.AluOpType
    Abs = mybir.ActivationFunctionType.Abs
    Copy = mybir.ActivationFunctionType.Copy

    cntA, mmA, entA = meta["cntA"], meta["mmA"], meta["entA"]
    cntC, mmC, entC = meta["cntC"], meta["mmC"], meta["entC"]
    emaxA, emaxC = meta["emaxA"], meta["emaxC"]
    emax = max(emaxA, emaxC)

    NQ = 4
    nc = bacc.Bacc(None, target_bir_lowering=False, debug=False,
                   num_devices=NC, num_swdge_queues=NQ)

    xq = [nc.dram_tensor(f"xq{c}", [CHUNK, F], bf16, kind="ExternalInput")
          for c in range(NCH)]
    gaA_d = nc.dram_tensor("gaA", [128, AW * NCH * 64], i16, kind="ExternalInput")
    sgA_d = nc.dram_tensor("sgA", [128, AW * NCH * emaxA], f32, kind="ExternalInput")
    gaC_d = nc.dram_tensor("gaC", [128, 2 * CW * 64], i16, kind="ExternalInput")
    sgC_d = nc.dram_tensor("sgC", [128, 2 * CW * emaxC], f32, kind="ExternalInput")
    xdt_d = nc.dram_tensor("xdt", [128, NODE_SH_P], bf16, kind="ExternalInput")
    wb1_d = nc.dram_tensor("wb1", [F, F], bf16, kind="ExternalInput")
    wb2_d = nc.dram_tensor("wb2", [F, F], bf16, kind="ExternalInput")
    wac1_d = nc.dram_tensor("wac1", [F, 2 * F], bf16, kind="ExternalInput")
    wac2_d = nc.dram_tensor("wac2", [F, 2 * F], bf16, kind="ExternalInput")
    bb_d = nc.dram_tensor("bb", [1, F], bf16, kind="ExternalInput")
    bac_d = nc.dram_tensor("bac", [1, 2 * F], bf16, kind="ExternalInput")
    iota_d = nc.dram_tensor("iota", [128, emax * 128], f32, kind="ExternalInput")
    ident_d = nc.dram_tensor("ident", [128, 128], bf16, kind="ExternalInput")
    out3 = nc.dram_tensor("out3", [NODE_SH_P, 3 * F], bf16, kind="ExternalOutput")

    xe = [nc.dram_tensor(f"xe{r}", [REG_ROWS[r], F], bf16) for r in range(2)]
    xt = [nc.dram_tensor(f"xt{r}", [TBL_ROWS[r], F], bf16) for r in range(2)]

    with tile.TileContext(nc) as tc:
        with (
            tc.tile_pool(name="cp", bufs=1) as cp,
            tc.tile_pool(name="dp", bufs=10) as dp,
            tc.tile_pool(name="ohp", bufs=6) as ohp,
            tc.tile_pool(name="sp", bufs=4) as sp,
            tc.tile_pool(name="psw", bufs=2, space="PSUM") as psw,
            tc.tile_pool(name="psd", bufs=2, space="PSUM") as psd,
        ):
            # ---- constants / preloads
            iota = cp.tile([128, emax * 128], f32, tag="iota")
            nc.sync.dma_start(iota[:], iota_d[:])
            identb = cp.tile([128, 128], bf16, tag="identb")
            nc.sync.dma_start(identb[:], ident_d[:])
            ones1 = cp.tile([1, F], bf16, tag="ones1")
            nc.vector.memset(ones1[:], 1.0)
            wb1s = cp.tile([F, F], bf16, tag="wb1s")
            nc.sync.dma_start(wb1s[:], wb1_d[:])
            wb2s = cp.tile([F, F], bf16, tag="wb2s")
            nc.sync.dma_start(wb2s[:], wb2_d[:])
            wac1s = cp.tile([F, 2 * F], bf16, tag="wac1s")
            nc.sync.dma_start(wac1s[:], wac1_d[:])
            wac2s = cp.tile([F, 2 * F], bf16, tag="wac2s")
            nc.sync.dma_start(wac2s[:], wac2_d[:])
            bbs = cp.tile([1, F], bf16, tag="bbs")
            nc.sync.dma_start(bbs[:], bb_d[:])
            bacs = cp.tile([1, 2 * F], bf16, tag="bacs")
            nc.sync.dma_start(bacs[:], bac_d[:])
            gaA_sb = cp.tile([128, AW * NCH * 64], i16, tag="gaA_sb")
            nc.scalar.dma_start(gaA_sb[:], gaA_d[:])
            sgA_sb = cp.tile([128, AW * NCH * emaxA], f32, tag="sgA_sb")
            nc.scalar.dma_start(sgA_sb[:], sgA_d[:])
            gaC_sb = cp.tile([128, 2 * CW * 64], i16, tag="gaC_sb")
            nc.scalar.dma_start(gaC_sb[:], gaC_d[:])
            sgC_sb = cp.tile([128, 2 * CW * emaxC], f32, tag="sgC_sb")
            nc.scalar.dma_start(sgC_sb[:], sgC_d[:])
            xdt_sb = cp.tile([128, NODE_SH_P], bf16, tag="xdt_sb")
            nc.sync.dma_start(xdt_sb[:], xdt_d[:])
            xv2sb = cp.tile([128, NODE_SH_P], bf16, tag="xv2sb")

            # bias_b broadcast tile (ones-matmul trick).  All psd tiles are
            # padded to a full 2KB PSUM bank: start_tensor_calc marks the
            # whole bank pending-zero, so co-resident tiles would corrupt
            # each other.
            psb = psd.tile([128, 512], f32, tag="pscen")
            nc.tensor.matmul(psb[:, :F], ones1[:], bbs[:], start=True, stop=True)
            bcb = cp.tile([128, F], f32, tag="bcb")
            nc.vector.tensor_copy(bcb[:], psb[:, :F])

            def build_oh(sg_sb, emax_s, k, nent, tag):
                # One DVE op builds every one-hot block of call k:
                # oh[p, (e, j)] = (iota[j] == segmb[p, k*emax+e])
                oh = ohp.tile([128, emax * 128], bf16, tag=tag)
                segb = sg_sb[:, k * emax_s:k * emax_s + nent]
                nc.vector.tensor_tensor(
                    oh[:, :nent * 128].rearrange("p (e j) -> p e j", e=nent),
                    iota[:, :nent * 128].rearrange("p (e j) -> p e j", e=nent),
                    segb.unsqueeze(-1).broadcast_to([128, nent, 128]),
                    op=Alu.is_equal)
                return oh

            # ---- stage A: Xe^T windows
            for w in range(AW):
                nsub = 4 if w < AW - 1 else 1
                wsz = nsub * 128
                ps = psw.tile([128, 512], f32, tag="psA")
                dats = {}
                ohs = {}
                for c in range(NCH):
                    k = w * 4 + c
                    if cntA[k] == 0:
                        continue
                    dat = dp.tile([128, 8, F], bf16, tag="dA")
                    nc.gpsimd.dma_gather(
                        dat[:], xq[c].ap(), gaA_sb[:, k * 64:(k + 1) * 64],
                        T, int(cntA[k]), F)
                    dats[c] = dat
                    if entA[k]:
                        ohs[c] = build_oh(sgA_sb, emaxA, k, len(entA[k]), "ohA")
                for (k, e0, b, s0, nsp, st, sp_) in mmA[w]:
                    nc.tensor.matmul(ps[:, s0 * 128:(s0 + nsp) * 128],
                                     dats[k % 4][:, b, :],
                                     ohs[k % 4][:, e0 * 128:(e0 + nsp) * 128],
                                     start=st, stop=sp_)
                xs = sp.tile([128, 512], bf16, tag="xs")
                nc.scalar.activation(xs[:, :wsz], ps[:, :wsz], Copy)
                pt = psd.tile([128, 1024], bf16, tag="pscen")
                for s in range(nsub):
                    nc.tensor.matmul(pt[:, s * 128:(s + 1) * 128],
                                     xs[:, s * 128:(s + 1) * 128], identb[:],
                                     is_transpose=True,
                                     start=(s == 0), stop=(s == nsub - 1))
                xo = sp.tile([128, 4, 128], bf16, tag="xo")
                nc.vector.tensor_copy(
                    xo[:, :nsub, :],
                    pt[:, :wsz].rearrange("p (s j) -> p s j", s=nsub))
                r, row = (0, w * 512) if w < 6 else (1, w * 512 - 3072)
                nc.scalar.dma_start(
                    xe[r][row:row + wsz, :].rearrange("(s p) j -> p s j", p=128),
                    xo[:, :nsub, :])
                if w == 5:
                    nc.gpsimd.collective_compute(
                        "AllGather", Alu.bypass,
                        replica_groups=[list(range(NC))],
                        ins=[xe[0].ap().opt()], outs=[xt[0].ap().opt()])
                if w == AW - 1:
                    nc.gpsimd.collective_compute(
                        "AllGather", Alu.bypass,
                        replica_groups=[list(range(NC))],
                        ins=[xe[1].ap().opt()], outs=[xt[1].ap().opt()])

            # ---- stage C: Xv2^T windows + fused dense head
            for r in range(2):
                for w in range(CW):
                    nsub = 4 if w < CW - 1 else 2
                    wsz = nsub * 128
                    k = r * CW + w
                    ps = psw.tile([128, 512], f32, tag="psC")
                    dat = dp.tile([128, 8, F], bf16, tag="dC")
                    nc.gpsimd.dma_gather(
                        dat[:], xt[r].ap(), gaC_sb[:, k * 64:(k + 1) * 64],
                        T, int(cntC[k]), F)
                    oh = build_oh(sgC_sb, emaxC, k, len(entC[k]), "ohC")
                    for (e0, b, s0, nsp, st, sp_) in mmC[k]:
                        nc.tensor.matmul(ps[:, s0 * 128:(s0 + nsp) * 128],
                                         dat[:, b, :],
                                         oh[:, e0 * 128:(e0 + nsp) * 128],
                                         start=st, stop=sp_)
                    sl = xv2sb[:, w * 512:w * 512 + wsz]
                    if r == 0:
                        nc.vector.tensor_copy(sl, ps[:, :wsz])
                        continue
                    nc.vector.tensor_add(sl, sl, ps[:, :wsz])
                    for g in range(w * 4, min(w * 4 + nsub, NGRP)):
                        xd = xdt_sb[:, g * 128:(g + 1) * 128]
                        v2 = xv2sb[:, g * 128:(g + 1) * 128]
                        axd = sp.tile([128, 128], bf16, tag="axd")
                        nc.scalar.activation(axd[:], xd, Abs)
                        av2 = sp.tile([128, 128], bf16, tag="av2")
                        nc.scalar.activation(av2[:], v2, Abs)
                        pc = psd.tile([128, 512], f32, tag="pscen")
                        nc.tensor.matmul(pc[:, :F], xd, wb1s[:], start=True, stop=False)
                        nc.tensor.matmul(pc[:, :F], v2, wb2s[:], start=False, stop=True)
                        pl = psd.tile([128, 512], f32, tag="pslr")
                        nc.tensor.matmul(pl[:, :2 * F], axd[:], wac1s[:],
                                         start=True, stop=False)
                        nc.tensor.matmul(pl[:, :2 * F], av2[:], wac2s[:],
                                         start=False, stop=False)
                        nc.tensor.matmul(pl[:, :2 * F], ones1[:], bacs[:],
                                         start=False, stop=True)
                        ot = sp.tile([128, 3 * F], bf16, tag="ot")
                        nc.vector.tensor_add(ot[:, 0:F], bcb[:], pc[:, :F])
                        nc.vector.tensor_add(
                            ot[:, F:3 * F].rearrange("p (h j) -> p h j", h=2),
                            ot[:, 0:F].unsqueeze(-2).broadcast_to([128, 2, F]),
                            pl[:, :2 * F].rearrange("p (h j) -> p h j", h=2))
                        rows = min(128, NODE_SH - g * 128)
                        nc.scalar.dma_start(out3[g * 128:g * 128 + rows, :],
                                            ot[:rows, :])

    # SWDGE queue assignment must match the DMASW semaphore lane the Tile
    # sem-assignment pass gave each gather (lanes rotate over Pool-engine
    # DMA insts in SCHEDULED order, which differs from emission order; a
    # lane's semaphore is locked to one queue).  queue = lane % NQ keeps
    # every lane on exactly one queue while spreading descriptor-gen work
    # across all 4 Q7 ucode workers.
    from concourse.tile_sem_assignment import PROC_NAME_TO_IDX
    idx2lane = {PROC_NAME_TO_IDX[f"DMASW{i}"]: i for i in range(8)}
    for insts in tc.ordered_instructions_by_block.values():
        for inst in insts:
            if isinstance(inst, mybir.InstDMAGatherAnt):
                lane = idx2lane.get(getattr(inst, "bass_scheduled_proc", -1))
                if lane is not None:
                    inst.queue_num = lane % NQ

    nc.compile()
    return nc


# ------------------------------------------------------------------- driver
def kernel(X, vertex, edges, X0, n_edges, w_b, w_a, w_c, b_b, b_a, b_c):
    from concourse.bass_utils import run_bass_kernel_spmd
    import ml_dtypes

    bf = ml_dtypes.bfloat16
    X = np.ascontiguousarray(np.asarray(X, dtype=np.float32))
    vertex = np.asarray(vertex).astype(np.int64)
    edges = np.asarray(edges).astype(np.int64)
    w_b = np.asarray(w_b, dtype=np.float32)
    w_a = np.asarray(w_a, dtype=np.float32)
    w_c = np.asarray(w_c, dtype=np.float32)
    b_b = np.asarray(b_b, dtype=np.float32).reshape(1, F)
    b_a = np.asarray(b_a, dtype=np.float32).reshape(1, F)
    b_c = np.asarray(b_c, dtype=np.float32).reshape(1, F)

    meta = _route(vertex, edges)
    if meta is None:
        return _numpy_fallback(X, vertex, edges, w_b, w_a, w_c, b_b, b_a, b_c)

    if _STATE.get("sig") != meta["sig"]:
        _STATE["nc"] = _build_program(meta)
        _STATE["sig"] = meta["sig"]
    nc = _STATE["nc"]

    Xbf = X.astype(bf)
    deg = np.bincount(vertex, minlength=N).astype(np.float32)
    XD = (X * deg[:, None]).astype(np.float32)

    emax = max(meta["emaxA"], meta["emaxC"])
    iota_np = np.ascontiguousarray(
        np.tile(np.arange(128, dtype=np.float32), (128, emax)))
    ident_np = np.ascontiguousarray(np.eye(128, dtype=np.float32).astype(bf))
    wb1 = np.ascontiguousarray(w_b[:F].astype(bf))
    wb2 = np.ascontiguousarray(w_b[F:].astype(bf))
    wac1 = np.ascontiguousarray(
        np.concatenate([-w_a[:F], w_c[:F]], axis=1).astype(bf))
    wac2 = np.ascontiguousarray(
        np.concatenate([-w_a[F:], w_c[F:]], axis=1).astype(bf))
    bb = np.ascontiguousarray(b_b.astype(bf))
    bac = np.ascontiguousarray(np.concatenate([-b_a, b_c], axis=1).astype(bf))

    in_maps = []
    for m in range(NC):
        xdm = np.zeros((128, NODE_SH_P), np.float32)
        xdm[:, :NODE_SH] = XD[m * NODE_SH:(m + 1) * NODE_SH].T
        im = {
            "gaA": _wrap16(meta["gaA"][m]),
            "sgA": _segmb_img(meta["sgA"][m], meta["entA"], meta["emaxA"]),
            "gaC": _wrap16(meta["gaC"][m]),
            "sgC": _segmb_img(meta["sgC"][m], meta["entC"], meta["emaxC"]),
            "xdt": np.ascontiguousarray(xdm.astype(bf)),
            "wb1": wb1, "wb2": wb2, "wac1": wac1, "wac2": wac2,
            "bb": bb, "bac": bac,
            "iota": iota_np, "ident": ident_np,
        }
        for c in range(NCH):
            im[f"xq{c}"] = np.ascontiguousarray(Xbf[c * CHUNK:(c + 1) * CHUNK])
        in_maps.append(im)

    res = run_bass_kernel_spmd(nc, in_maps, list(range(NC)))
    out = np.concatenate(
        [np.asarray(res.results[m]["out3"])[:NODE_SH].astype(np.float32)
         for m in range(NC)], axis=0)
    return (np.ascontiguousarray(out[:, 0:F]),
            np.ascontiguousarray(out[:, F:2 * F]),
            np.ascontiguousarray(out[:, 2 * F:3 * F]))



# revision 2
# speedup vs baseline: 1.0476x; 1.0476x over previous
"""Trainium2 Bass kernel v2 for nn_CrispToFuzzyConv (hypergraph message passing).

Math (see reference):
  Xe   = segment_sum(X[vertex], edges, E)                 # round 1
  Xv   = concat([deg * X, Xv2]),  Xv2 = segment_sum(Xe[edges], vertex, N)
  center = Xv @ w_b + b_b
  HL = center - (|Xv| @ w_a + b_a)
  HR = center + (|Xv| @ w_c + b_c)

v2 strategy (vs v1 which used dma_scatter_add):
  - NO scatter-adds.  Segment sums are computed as one-hot matmuls on the
    PE: gathered token rows [128tok x F] are contracted against on-chip
    one-hot matrices.  All of one gather call's one-hot blocks are built
    with a SINGLE DVE tensor_tensor:
        oh[p, (e, j)] = (iota_rep[j] == segmb[p, e])     # stride-0 bcast
    where segmb = segid - (window*512 + sub*128) is precomputed host-side
    per mm entry (DVE instruction overhead ~0.5-1us dwarfs the per-element
    cost, so batching 5-15 builds into one op is ~10x cheaper).
  - For stage A the PSUM window is Xe^T [F x 512 edge-cols]; for stage C
    it is Xv2^T [F x 512 node-cols] which feeds the dense head directly
    as matmul lhsT (no transposes in the dense path; the host supplies
    deg-scaled X^T).
  - Everything is bf16 on the wire (gather rows are 256B packets, the
    AllGather payload halves); PSUM accumulates in f32.
  - SPMD: one program for all 8 cores.  The schedule is static, sized by
    the max token count over cores per (window, chunk/region) cell; each
    cell is one 1024-slot gather call whose runtime count (num_idxs_reg)
    is that max, so padding costs ~3% packets, not 30%.
  - Stage C accumulates region passes into an SBUF-resident Xv2 table
    (24.5KB/partition) so PSUM windows stay short-lived; the dense head
    runs fused per 128-node group right after its window closes.
  - Two AllGathers (edge regions) pipeline behind stage A / stage C
    descriptor generation.

Hardware constraints baked in (v1 + this session's trace/sim analysis):
  - dma_gather idx dtype is int16 -> gather tables <= 32767 rows: X is
    split in 4 chunks of 25000, Xe tables in 2 regions (24576/25600 rows)
  - elem_size_bytes % 256 == 0 -> bf16 F=128 rows (256B) are legal
  - gather output layout: token i -> partition i%128, col-block i//128,
    which is exactly the PE contraction layout
  - SWDGE requires num_idxs_reg == #(idx >= 0) and valid idxs contiguous
    from slot 0 (trailing -1 pads); slots past the count are never
    written, so the boundary block's tail is zero-filled via a small DMA
    (engine ops can't start at arbitrary partitions)
  - PSUM accumulation "zero regions" are 2KB (a whole bank): start/stop
    flags bracket a window's full mm list, and every PSUM tile is padded
    to a full bank so start_tensor_calc can't corrupt a neighbor
  - the Tile sem-assignment pass locks each DMASW semaphore lane to one
    SWDGE queue; lanes rotate over Pool-engine DMA insts in SCHEDULED
    order, so queue_num is assigned AFTER scheduling as lane % 4
  - per-instruction overhead (~0.3-1us) dominates small ops: one-hot
    builds are batched per call, DMA triggers are spread across sync /
    scalar engines
"""

import numpy as np

# ---------------------------------------------------------------- constants
N = 100000
E = 50000
NNZ = 300000
F = 128
NC = 8

NODE_SH = 12500              # nodes per core
NODE_SH_P = 12544            # 98 groups of 128
NGRP = 98
CW = 25                      # 512-node windows (last = 256)

EDGE_SH = 6250               # edges per core
EDGE_SH_P = 6272             # 49 subwindows of 128
AW = 13                      # 512-edge windows (last = 128)
NCH = 4
CHUNK = 25000                # X chunk rows (int16 gather limit)

REG_SPLIT = 3072             # local edge rows [0,3072) -> region 0
REG_ROWS = (3072, 3200)      # padded local rows per region
TBL_ROWS = (NC * 3072, NC * 3200)

T = 1024                     # slots per gather call (8 blocks of 128)

_STATE = {}


# ---------------------------------------------------------------- host side
def _wrap16(idx):
    """[ncall, 1024] int -> [128, ncall*64] int16 SBUF image (idx i at
    partition i%16, col i//16; replicated across the 8 groups of 16)."""
    ncall = idx.shape[0]
    t = idx.reshape(ncall, 64, 16).transpose(0, 2, 1).astype(np.int16)
    t = np.tile(t, (1, 8, 1))                      # [ncall, 128, 64]
    return np.ascontiguousarray(t.transpose(1, 0, 2).reshape(128, ncall * 64))


def _segmb_img(sg, entries, emax):
    """Per-entry biased segid image: [128, ncall*emax] f32 where column
    k*emax+e holds segid(block(e)) - base(e) for call k (never a valid
    one-hot match for pad slots: segid -1 -> negative)."""
    ncall = sg.shape[0]
    img = np.full((128, ncall * emax), -3.0e6, np.float32)
    for k in range(ncall):
        for e, (b, base) in enumerate(entries[k]):
            img[:, k * emax + e] = sg[k, b * 128:(b + 1) * 128] - base
    return np.ascontiguousarray(img)


def _build_stream(owner, cell, gidx, segid, ncells):
    """Pack tokens into per-(core, cell) sorted slot arrays."""
    ga = np.zeros((NC, ncells, T), np.int64)
    sg = np.full((NC, ncells, T), -1.0, np.float32)
    cnt = np.zeros((NC, ncells), np.int64)
    for m in range(NC):
        s = np.nonzero(owner == m)[0]
        o = np.lexsort((segid[s], cell[s]))
        s = s[o]
        cs = cell[s]
        bounds = np.searchsorted(cs, np.arange(ncells + 1))
        for k in range(ncells):
            lo, hi = int(bounds[k]), int(bounds[k + 1])
            n = hi - lo
            if n > T:
                return None
            ga[m, k, :n] = gidx[s[lo:hi]]
            sg[m, k, :n] = segid[s[lo:hi]]
            cnt[m, k] = n
    return ga, sg, cnt


def _cell_entries(sg_all, k, cnt_max, wbase, nsub):
    """Static (block, sub) entry list for one cell, from the union of all
    cores' segids.  Returns None if a segid falls outside the window."""
    entries = []
    nb = -(-int(cnt_max) // 128)
    for b in range(nb):
        vals = sg_all[:, k, b * 128:(b + 1) * 128].ravel()
        vals = vals[vals >= 0].astype(np.int64)
        if len(vals) == 0:
            continue
        for sub in np.unique(vals // 128):
            s = int(sub) - wbase // 128
            if s < 0 or s >= nsub:
                return None
            entries.append((b, s))
    return entries


def _route(vertex, edges):
    eo = edges // EDGE_SH
    le = edges % EDGE_SH
    vo = vertex // NODE_SH
    lv = vertex % NODE_SH
    ch = vertex // CHUNK
    reg = (le >= REG_SPLIT).astype(np.int64)
    trow = np.where(reg == 0, eo * REG_ROWS[0] + le,
                    eo * REG_ROWS[1] + (le - REG_SPLIT))

    rA = _build_stream(eo, (le // 512) * 4 + ch, vertex - ch * CHUNK,
                       le.astype(np.float32), AW * NCH)
    if rA is None:
        return None
    gaA, sgA, cntA = rA
    rC = _build_stream(vo, reg * CW + lv // 512, trow,
                       lv.astype(np.float32), 2 * CW)
    if rC is None:
        return None
    gaC, sgC, cntC = rC

    # Round static counts up to a multiple of 128 (idx-0 pad slots, segid
    # -1): every mm-read block is then fully DMA-written, so no stale-slot
    # zero-fill (and its cross-engine dep hop) is needed.
    cntA_max = np.minimum(-(-cntA.max(axis=0) // 128) * 128, T)
    cntC_max = np.minimum(-(-cntC.max(axis=0) // 128) * 128, T)
    # Slots beyond the static (max-over-cores) count must be NEGATIVE: the
    # SWDGE ucode requires num_idxs_reg == #(idx >= 0) and stops at the
    # last non-negative slot.  Slots in [cnt_m, cnt_max) keep idx 0 (valid
    # row; their segid is -1 so the one-hot zeroes them out).
    for k in range(AW * NCH):
        gaA[:, k, int(cntA_max[k]):] = -1
    for k in range(2 * CW):
        gaC[:, k, int(cntC_max[k]):] = -1

    def merge_runs(ents):
        # consecutive entries of the same block with consecutive subs are
        # one matmul (contiguous oh slice -> contiguous psum cols)
        runs = []
        i = 0
        while i < len(ents):
            b, s = ents[i]
            n = 1
            while i + n < len(ents) and ents[i + n] == (b, s + n):
                n += 1
            runs.append((i, b, s, n))
            i += n
        return runs

    # Entry lists per call: (block, sub) plus base for segmb biasing.
    entA = []          # per call: [(b, base)]
    mmA = []           # per window: [(cell, e0, block, s0, nsp, st, sp)]
    for w in range(AW):
        nsub = 4 if w < AW - 1 else 1
        wlist = []
        subs_seen = set()
        for c in range(NCH):
            k = w * 4 + c
            ents = _cell_entries(sgA, k, cntA_max[k], w * 512, nsub)
            if ents is None:
                return None
            entA.append([(b, w * 512 + s * 128) for (b, s) in ents])
            subs_seen |= set(s for (_, s) in ents)
            wlist += [(k, e0, b, s0, n) for (e0, b, s0, n) in merge_runs(ents)]
        if subs_seen != set(range(nsub)):
            return None
        mmA.append([(k, e0, b, s0, n, i == 0, i == len(wlist) - 1)
                    for i, (k, e0, b, s0, n) in enumerate(wlist)])

    entC = []
    mmC = []           # per cell k: [(e0, block, s0, nsp, st, sp)]
    for r in range(2):
        for w in range(CW):
            nsub = 4 if w < CW - 1 else 2
            k = r * CW + w
            ents = _cell_entries(sgC, k, cntC_max[k], w * 512, nsub)
            if ents is None or not ents:
                return None
            if set(s for (_, s) in ents) != set(range(nsub)):
                return None
            entC.append([(b, w * 512 + s * 128) for (b, s) in ents])
            runs = merge_runs(ents)
            mmC.append([(e0, b, s0, n, i == 0, i == len(runs) - 1)
                        for i, (e0, b, s0, n) in enumerate(runs)])

    emaxA = max(len(x) for x in entA)
    emaxC = max(len(x) for x in entC)
    sig = repr((tuple(cntA_max), tuple(cntC_max), mmA, mmC, emaxA, emaxC))
    return dict(gaA=gaA, sgA=sgA, cntA=cntA_max, mmA=mmA, entA=entA,
                gaC=gaC, sgC=sgC, cntC=cntC_max, mmC=mmC, entC=entC,
                emaxA=emaxA, emaxC=emaxC, sig=sig)


def _numpy_fallback(X, vertex, edges, w_b, w_a, w_c, b_b, b_a, b_c):
    Xe = np.zeros((E, F), np.float32)
    np.add.at(Xe, edges, X[vertex])
    Xv2 = np.zeros((N, F), np.float32)
    np.add.at(Xv2, vertex, Xe[edges])
    deg = np.bincount(vertex, minlength=N).astype(np.float32)[:, None]
    Xv = np.concatenate([deg * X, Xv2], axis=1)
    center = Xv @ w_b + b_b
    aXv = np.abs(Xv)
    return (center.astype(np.float32),
            (center - (aXv @ w_a + b_a)).astype(np.float32),
            (center + (aXv @ w_c + b_c)).astype(np.float32))


# ------------------------------------------------------------- bass program
def _build_program(meta):
    from concourse import bacc, tile
    import concourse.mybir as mybir

    f32 = mybir.dt.float32
    bf16 = mybir.dt.bfloat16
    i16 = mybir.dt.int16
    Alu = mybir.AluOpType
    Abs = mybir.ActivationFunctionType.Abs
    Copy = mybir.ActivationFunctionType.Copy

    cntA, mmA, entA = meta["cntA"], meta["mmA"], meta["entA"]
    cntC, mmC, entC = meta["cntC"], meta["mmC"], meta["entC"]
    emaxA, emaxC = meta["emaxA"], meta["emaxC"]
    emax = max(emaxA, emaxC)

    NQ = 4
    nc = bacc.Bacc(None, target_bir_lowering=False, debug=False,
                   num_devices=NC, num_swdge_queues=NQ)

    xq = [nc.dram_tensor(f"xq{c}", [CHUNK, F], bf16, kind="ExternalInput")
          for c in range(NCH)]
    gaA_d = nc.dram_tensor("gaA", [128, AW * NCH * 64], i16, kind="ExternalInput")
    sgA_d = nc.dram_tensor("sgA", [128, AW * NCH * emaxA], f32, kind="ExternalInput")
    gaC_d = nc.dram_tensor("gaC", [128, 2 * CW * 64], i16, kind="ExternalInput")
    sgC_d = nc.dram_tensor("sgC", [128, 2 * CW * emaxC], f32, kind="ExternalInput")
    xdt_d = nc.dram_tensor("xdt", [128, NODE_SH_P], bf16, kind="ExternalInput")
    wb1_d = nc.dram_tensor("wb1", [F, F], bf16, kind="ExternalInput")
    wb2_d = nc.dram_tensor("wb2", [F, F], bf16, kind="ExternalInput")
    wac1_d = nc.dram_tensor("wac1", [F, 2 * F], bf16, kind="ExternalInput")
    wac2_d = nc.dram_tensor("wac2", [F, 2 * F], bf16, kind="ExternalInput")
    bb_d = nc.dram_tensor("bb", [1, F], bf16, kind="ExternalInput")
    bac_d = nc.dram_tensor("bac", [1, 2 * F], bf16, kind="ExternalInput")
    iota_d = nc.dram_tensor("iota", [128, emax * 128], f32, kind="ExternalInput")
    ident_d = nc.dram_tensor("ident", [128, 128], bf16, kind="ExternalInput")
    out3 = nc.dram_tensor("out3", [NODE_SH_P, 3 * F], bf16, kind="ExternalOutput")

    xe = [nc.dram_tensor(f"xe{r}", [REG_ROWS[r], F], bf16) for r in range(2)]
    xt = [nc.dram_tensor(f"xt{r}", [TBL_ROWS[r], F], bf16, addr_space="Shared")
          for r in range(2)]

    with tile.TileContext(nc) as tc:
        with (
            tc.tile_pool(name="cp", bufs=1) as cp,
            tc.tile_pool(name="dp", bufs=10) as dp,
            tc.tile_pool(name="ohp", bufs=6) as ohp,
            tc.tile_pool(name="sp", bufs=4) as sp,
            tc.tile_pool(name="psw", bufs=2, space="PSUM") as psw,
            tc.tile_pool(name="psd", bufs=2, space="PSUM") as psd,
        ):
            # ---- constants / preloads
            iota = cp.tile([128, emax * 128], f32, tag="iota")
            nc.sync.dma_start(iota[:], iota_d[:])
            identb = cp.tile([128, 128], bf16, tag="identb")
            nc.sync.dma_start(identb[:], ident_d[:])
            ones1 = cp.tile([1, F], bf16, tag="ones1")
            nc.vector.memset(ones1[:], 1.0)
            wb1s = cp.tile([F, F], bf16, tag="wb1s")
            nc.sync.dma_start(wb1s[:], wb1_d[:])
            wb2s = cp.tile([F, F], bf16, tag="wb2s")
            nc.sync.dma_start(wb2s[:], wb2_d[:])
            wac1s = cp.tile([F, 2 * F], bf16, tag="wac1s")
            nc.sync.dma_start(wac1s[:], wac1_d[:])
            wac2s = cp.tile([F, 2 * F], bf16, tag="wac2s")
            nc.sync.dma_start(wac2s[:], wac2_d[:])
            bbs = cp.tile([1, F], bf16, tag="bbs")
            nc.sync.dma_start(bbs[:], bb_d[:])
            bacs = cp.tile([1, 2 * F], bf16, tag="bacs")
            nc.sync.dma_start(bacs[:], bac_d[:])
            gaA_sb = cp.tile([128, AW * NCH * 64], i16, tag="gaA_sb")
            nc.scalar.dma_start(gaA_sb[:], gaA_d[:])
            sgA_sb = cp.tile([128, AW * NCH * emaxA], f32, tag="sgA_sb")
            nc.scalar.dma_start(sgA_sb[:], sgA_d[:])
            gaC_sb = cp.tile([128, 2 * CW * 64], i16, tag="gaC_sb")
            nc.scalar.dma_start(gaC_sb[:], gaC_d[:])
            sgC_sb = cp.tile([128, 2 * CW * emaxC], f32, tag="sgC_sb")
            nc.scalar.dma_start(sgC_sb[:], sgC_d[:])
            xdt_sb = cp.tile([128, NODE_SH_P], bf16, tag="xdt_sb")
            nc.sync.dma_start(xdt_sb[:], xdt_d[:])
            xv2sb = cp.tile([128, NODE_SH_P], bf16, tag="xv2sb")

            # bias_b broadcast tile (ones-matmul trick).  All psd tiles are
            # padded to a full 2KB PSUM bank: start_tensor_calc marks the
            # whole bank pending-zero, so co-resident tiles would corrupt
            # each other.
            psb = psd.tile([128, 512], f32, tag="pscen")
            nc.tensor.matmul(psb[:, :F], ones1[:], bbs[:], start=True, stop=True)
            bcb = cp.tile([128, F], f32, tag="bcb")
            nc.vector.tensor_copy(bcb[:], psb[:, :F])

            def build_oh(sg_sb, emax_s, k, nent, tag):
                # One DVE op builds every one-hot block of call k:
                # oh[p, (e, j)] = (iota[j] == segmb[p, k*emax+e])
                oh = ohp.tile([128, emax * 128], bf16, tag=tag)
                segb = sg_sb[:, k * emax_s:k * emax_s + nent]
                nc.vector.tensor_tensor(
                    oh[:, :nent * 128].rearrange("p (e j) -> p e j", e=nent),
                    iota[:, :nent * 128].rearrange("p (e j) -> p e j", e=nent),
                    segb.unsqueeze(-1).broadcast_to([128, nent, 128]),
                    op=Alu.is_equal)
                return oh

            # ---- stage A: Xe^T windows
            for w in range(AW):
                nsub = 4 if w < AW - 1 else 1
                wsz = nsub * 128
                ps = psw.tile([128, 512], f32, tag="psA")
                dats = {}
                ohs = {}
                for c in range(NCH):
                    k = w * 4 + c
                    if cntA[k] == 0:
                        continue
                    dat = dp.tile([128, 8, F], bf16, tag="dA")
                    nc.gpsimd.dma_gather(
                        dat[:], xq[c].ap(), gaA_sb[:, k * 64:(k + 1) * 64],
                        T, int(cntA[k]), F)
                    dats[c] = dat
                    if entA[k]:
                        ohs[c] = build_oh(sgA_sb, emaxA, k, len(entA[k]), "ohA")
                for (k, e0, b, s0, nsp, st, sp_) in mmA[w]:
                    nc.tensor.matmul(ps[:, s0 * 128:(s0 + nsp) * 128],
                                     dats[k % 4][:, b, :],
                                     ohs[k % 4][:, e0 * 128:(e0 + nsp) * 128],
                                     start=st, stop=sp_)
                xs = sp.tile([128, 512], bf16, tag="xs")
                nc.scalar.activation(xs[:, :wsz], ps[:, :wsz], Copy)
                pt = psd.tile([128, 1024], bf16, tag="pscen")
                for s in range(nsub):
                    nc.tensor.matmul(pt[:, s * 128:(s + 1) * 128],
                                     xs[:, s * 128:(s + 1) * 128], identb[:],
                                     is_transpose=True,
                                     start=(s == 0), stop=(s == nsub - 1))
                xo = sp.tile([128, 4, 128], bf16, tag="xo")
                nc.vector.tensor_copy(
                    xo[:, :nsub, :],
                    pt[:, :wsz].rearrange("p (s j) -> p s j", s=nsub))
                r, row = (0, w * 512) if w < 6 else (1, w * 512 - 3072)
                nc.scalar.dma_start(
                    xe[r][row:row + wsz, :].rearrange("(s p) j -> p s j", p=128),
                    xo[:, :nsub, :])
                if w == 5:
                    nc.gpsimd.collective_compute(
                        "AllGather", Alu.bypass,
                        replica_groups=[list(range(NC))],
                        ins=[xe[0].ap().opt()], outs=[xt[0].ap().opt()])
                if w == AW - 1:
                    nc.gpsimd.collective_compute(
                        "AllGather", Alu.bypass,
                        replica_groups=[list(range(NC))],
                        ins=[xe[1].ap().opt()], outs=[xt[1].ap().opt()])

            # ---- stage C: Xv2^T windows + fused dense head
            for r in range(2):
                for w in range(CW):
                    nsub = 4 if w < CW - 1 else 2
                    wsz = nsub * 128
                    k = r * CW + w
                    ps = psw.tile([128, 512], f32, tag="psC")
                    dat = dp.tile([128, 8, F], bf16, tag="dC")
                    nc.gpsimd.dma_gather(
                        dat[:], xt[r].ap(), gaC_sb[:, k * 64:(k + 1) * 64],
                        T, int(cntC[k]), F)
                    oh = build_oh(sgC_sb, emaxC, k, len(entC[k]), "ohC")
                    for (e0, b, s0, nsp, st, sp_) in mmC[k]:
                        nc.tensor.matmul(ps[:, s0 * 128:(s0 + nsp) * 128],
                                         dat[:, b, :],
                                         oh[:, e0 * 128:(e0 + nsp) * 128],
                                         start=st, stop=sp_)
                    sl = xv2sb[:, w * 512:w * 512 + wsz]
                    if r == 0:
                        nc.vector.tensor_copy(sl, ps[:, :wsz])
                        continue
                    nc.vector.tensor_add(sl, sl, ps[:, :wsz])
                    for g in range(w * 4, min(w * 4 + nsub, NGRP)):
                        xd = xdt_sb[:, g * 128:(g + 1) * 128]
                        v2 = xv2sb[:, g * 128:(g + 1) * 128]
                        axd = sp.tile([128, 128], bf16, tag="axd")
                        nc.scalar.activation(axd[:], xd, Abs)
                        av2 = sp.tile([128, 128], bf16, tag="av2")
                        nc.scalar.activation(av2[:], v2, Abs)
                        pc = psd.tile([128, 512], f32, tag="pscen")
                        nc.tensor.matmul(pc[:, :F], xd, wb1s[:], start=True, stop=False)
                        nc.tensor.matmul(pc[:, :F], v2, wb2s[:], start=False, stop=True)
                        pl = psd.tile([128, 512], f32, tag="pslr")
                        nc.tensor.matmul(pl[:, :2 * F], axd[:], wac1s[:],
                                         start=True, stop=False)
                        nc.tensor.matmul(pl[:, :2 * F], av2[:], wac2s[:],
                                         start=False, stop=False)
                        nc.tensor.matmul(pl[:, :2 * F], ones1[:], bacs[:],
                                         start=False, stop=True)
                        ot = sp.tile([128, 3 * F], bf16, tag="ot")
                        nc.vector.tensor_add(ot[:, 0:F], bcb[:], pc[:, :F])
                        nc.vector.tensor_add(
                            ot[:, F:3 * F].rearrange("p (h j) -> p h j", h=2),
                            ot[:, 0:F].unsqueeze(-2).broadcast_to([128, 2, F]),
                            pl[:, :2 * F].rearrange("p (h j) -> p h j", h=2))
                        rows = min(128, NODE_SH - g * 128)
                        nc.scalar.dma_start(out3[g * 128:g * 128 + rows, :],
                                            ot[:rows, :])

    # SWDGE queue assignment must match the DMASW semaphore lane the Tile
    # sem-assignment pass gave each gather (lanes rotate over Pool-engine
    # DMA insts in SCHEDULED order, which differs from emission order; a
    # lane's semaphore is locked to one queue).  queue = lane % NQ keeps
    # every lane on exactly one queue while spreading descriptor-gen work
    # across all 4 Q7 ucode workers.
    from concourse.tile_sem_assignment import PROC_NAME_TO_IDX
    idx2lane = {PROC_NAME_TO_IDX[f"DMASW{i}"]: i for i in range(8)}
    for insts in tc.ordered_instructions_by_block.values():
        for inst in insts:
            if isinstance(inst, mybir.InstDMAGatherAnt):
                lane = idx2lane.get(getattr(inst, "bass_scheduled_proc", -1))
                if lane is not None:
                    inst.queue_num = lane % NQ

    nc.compile()
    return nc


# ------------------------------------------------------------------- driver
def kernel(X, vertex, edges, X0, n_edges, w_b, w_a, w_c, b_b, b_a, b_c):
    from concourse.bass_utils import run_bass_kernel_spmd
    import ml_dtypes

    bf = ml_dtypes.bfloat16
    X = np.ascontiguousarray(np.asarray(X, dtype=np.float32))
    vertex = np.asarray(vertex).astype(np.int64)
    edges = np.asarray(edges).astype(np.int64)
    w_b = np.asarray(w_b, dtype=np.float32)
    w_a = np.asarray(w_a, dtype=np.float32)
    w_c = np.asarray(w_c, dtype=np.float32)
    b_b = np.asarray(b_b, dtype=np.float32).reshape(1, F)
    b_a = np.asarray(b_a, dtype=np.float32).reshape(1, F)
    b_c = np.asarray(b_c, dtype=np.float32).reshape(1, F)

    meta = _route(vertex, edges)
    if meta is None:
        return _numpy_fallback(X, vertex, edges, w_b, w_a, w_c, b_b, b_a, b_c)

    if _STATE.get("sig") != meta["sig"]:
        _STATE["nc"] = _build_program(meta)
        _STATE["sig"] = meta["sig"]
    nc = _STATE["nc"]

    Xbf = X.astype(bf)
    deg = np.bincount(vertex, minlength=N).astype(np.float32)
    XD = (X * deg[:, None]).astype(np.float32)

    emax = max(meta["emaxA"], meta["emaxC"])
    iota_np = np.ascontiguousarray(
        np.tile(np.arange(128, dtype=np.float32), (128, emax)))
    ident_np = np.ascontiguousarray(np.eye(128, dtype=np.float32).astype(bf))
    wb1 = np.ascontiguousarray(w_b[:F].astype(bf))
    wb2 = np.ascontiguousarray(w_b[F:].astype(bf))
    wac1 = np.ascontiguousarray(
        np.concatenate([-w_a[:F], w_c[:F]], axis=1).astype(bf))
    wac2 = np.ascontiguousarray(
        np.concatenate([-w_a[F:], w_c[F:]], axis=1).astype(bf))
    bb = np.ascontiguousarray(b_b.astype(bf))
    bac = np.ascontiguousarray(np.concatenate([-b_a, b_c], axis=1).astype(bf))

    in_maps = []
    for m in range(NC):
        xdm = np.zeros((128, NODE_SH_P), np.float32)
        xdm[:, :NODE_SH] = XD[m * NODE_SH:(m + 1) * NODE_SH].T
        im = {
            "gaA": _wrap16(meta["gaA"][m]),
            "sgA": _segmb_img(meta["sgA"][m], meta["entA"], meta["emaxA"]),
            "gaC": _wrap16(meta["gaC"][m]),
            "sgC": _segmb_img(meta["sgC"][m], meta["entC"], meta["emaxC"]),
            "xdt": np.ascontiguousarray(xdm.astype(bf)),
            "wb1": wb1, "wb2": wb2, "wac1": wac1, "wac2": wac2,
            "bb": bb, "bac": bac,
            "iota": iota_np, "ident": ident_np,
        }
        for c in range(NCH):
            im[f"xq{c}"] = np.ascontiguousarray(Xbf[c * CHUNK:(c + 1) * CHUNK])
        in_maps.append(im)

    res = run_bass_kernel_spmd(nc, in_maps, list(range(NC)))
    out = np.concatenate(
        [np.asarray(res.results[m]["out3"])[:NODE_SH].astype(np.float32)
         for m in range(NC)], axis=0)
    return (np.ascontiguousarray(out[:, 0:F]),
            np.ascontiguousarray(out[:, F:2 * F]),
            np.ascontiguousarray(out[:, 2 * F:3 * F]))



# revision 9
# speedup vs baseline: 1.5901x; 1.5179x over previous
"""Trainium2 Bass kernel v3 for nn_CrispToFuzzyConv (hypergraph message passing).

Math (see reference):
  Xe   = segment_sum(X[vertex], edges, E)                 # round 1
  Xv   = concat([deg * X, Xv2]),  Xv2 = segment_sum(Xe[edges], vertex, N)
  center = Xv @ w_b + b_b
  HL = center - (|Xv| @ w_a + b_a)
  HR = center + (|Xv| @ w_c + b_c)

v3 strategy (vs v2):
  - Stage A consumes a HOST-PRE-GATHERED token stream (X[vertex] rows sorted
    by (edge window, edge id), already in the [128, block, F] PE layout).
    This removes all 52 stage-A SWDGE gather calls (the v2 trace showed the
    gpsimd/Pool engine 65% busy, ~3.5us per dma_gather call) and the 4-chunk
    cell split; stage A becomes plain sequential DMA + one-hot matmuls.
  - Stage C keeps device gathers (Xe is device-computed) but decouples the
    gather-call granularity from the PSUM window: one call per (region,
    supercell of 5 node windows) = 10 calls total instead of 50.  mm entries
    are emitted grouped by PSUM window so only one window accumulates at a
    time per ring slot.
  - AllGather tables are addr_space="Shared" (fast HBM-HBM collective path;
    measured cc_op 225us -> 192us on v2).
  - Dense head runs transposed: out^T[f_out, node] = w^T-stationary matmuls
    with 512-node moving operands (4x fewer, 4x larger PE instructions than
    v2's per-128-node form), biases folded into the PSUM->SBUF copy via
    scalar Identity activation with a per-partition bias column, |deg*X|
    precomputed on host.  The host transposes the [128, 3*nodes] result.
  - PSUM: psA ring 2 banks (shared with the transpose tiles), psC ring 2,
    head ring 4 (3 tiles/window -> 1.33 windows in flight) = 8 banks.

Hardware constraints baked in (from v1/v2 sessions):
  - dma_gather idx dtype is int16 -> gather tables <= 32767 rows: Xe tables
    split in 2 regions (24576/25600 rows)
  - elem_size_bytes % 256 == 0 -> bf16 F=128 rows (256B) are legal
  - gather output layout: token i -> partition i%128, col-block i//128
  - SWDGE requires num_idxs_reg == #(idx >= 0); every slot of a call is kept
    valid (counts rounded up to 128 with idx-0/segid-(-1) pad slots) so all
    gathered blocks are fully DMA-written
  - PSUM "zero regions" are 2KB banks; every PSUM tile is padded to a bank
  - the Tile sem-assignment pass locks each DMASW semaphore lane to one
    SWDGE queue; queue_num is assigned AFTER scheduling as lane % 4
"""

import numpy as np

# ---------------------------------------------------------------- constants
N = 100000
E = 50000
NNZ = 300000
F = 128
NC = 8

NODE_SH = 12500              # nodes per core
NODE_SH_P = 12544            # 98 subs of 128
CW = 25                      # 512-node windows (last = 256)
NSC = 5                      # supercells per region (5 windows each)
SCN = 2560                   # nodes per supercell
TC_CAP = 6144                # slot cap per stage-C gather call

EDGE_SH = 6250               # edges per core
AW = 13                      # 512-edge windows (last = 128)
TA_CAP = 4096                # slot cap per stage-A window stream

REG_SPLIT = 3072             # local edge rows [0,3072) -> region 0
REG_ROWS = (3072, 3200)      # padded local rows per region
TBL_ROWS = (NC * 3072, NC * 3200)

GCALL = 1024                 # max slots per dma_gather sub-call (ucode-proven)

_STATE = {}


# ---------------------------------------------------------------- host side
def _wrap16v(idx_cells):
    """list of per-cell [T_k] int arrays -> [128, sum(T_k/16)] int16 image
    (idx i at partition i%16, col co_k + i//16; replicated across the 8
    groups of 16)."""
    cols = sum(a.shape[0] // 16 for a in idx_cells)
    img = np.zeros((128, cols), np.int16)
    co = 0
    for a in idx_cells:
        t = a.reshape(-1, 16).T.astype(np.int16)      # [16, T/16]
        img[:, co:co + t.shape[1]] = np.tile(t, (8, 1))
        co += t.shape[1]
    return np.ascontiguousarray(img)


def _segmb_img(sg, entries, emax):
    """Per-entry biased segid image: [128, ncell*emax] f32 where column
    k*emax+e holds segid(block(e)) - base(e) for cell k (never a valid
    one-hot match for pad slots: segid -1 -> negative)."""
    ncell = len(entries)
    img = np.full((128, ncell * emax), -3.0e6, np.float32)
    for k in range(ncell):
        for e, (b, base) in enumerate(entries[k]):
            img[:, k * emax + e] = sg[k, b * 128:(b + 1) * 128] - base
    return np.ascontiguousarray(img)


def _build_stream(owner, cell, gidx, segid, ncells, T):
    """Pack tokens into per-(core, cell) sorted slot arrays."""
    ga = np.zeros((NC, ncells, T), np.int64)
    sg = np.full((NC, ncells, T), -1.0, np.float32)
    cnt = np.zeros((NC, ncells), np.int64)
    for m in range(NC):
        s = np.nonzero(owner == m)[0]
        o = np.lexsort((segid[s], cell[s]))
        s = s[o]
        cs = cell[s]
        bounds = np.searchsorted(cs, np.arange(ncells + 1))
        for k in range(ncells):
            lo, hi = int(bounds[k]), int(bounds[k + 1])
            n = hi - lo
            if n > T:
                return None
            ga[m, k, :n] = gidx[s[lo:hi]]
            sg[m, k, :n] = segid[s[lo:hi]]
            cnt[m, k] = n
    return ga, sg, cnt


def _cell_entries(sg_all, k, cnt_max, sub_base, nsub):
    """Static (block, sub) entry list for one cell, from the union of all
    cores' segids.  subs relative to sub_base; None if out of range or if
    any sub in [0, nsub) has no tokens (its PSUM cols would stay unwritten).
    """
    entries = []
    subs_seen = set()
    nb = -(-int(cnt_max) // 128)
    for b in range(nb):
        vals = sg_all[:, k, b * 128:(b + 1) * 128].ravel()
        vals = vals[vals >= 0].astype(np.int64)
        if len(vals) == 0:
            continue
        for sub in np.unique(vals // 128):
            s = int(sub) - sub_base
            if s < 0 or s >= nsub:
                return None
            entries.append((b, s))
            subs_seen.add(s)
    if subs_seen != set(range(nsub)):
        return None
    return entries


def _runs(ents):
    """One matmul per (block, sub) entry.  Runs are NOT merged across sub
    boundaries: the PSUM zero-region is the whole 2KB bank, so an mm view
    must be uniformly pending-zero (first touch of a sub after the group's
    single start_tensor_calc) or uniformly written (accumulation) -- a
    merged multi-sub view would mix the two states."""
    return [(i, b, s, 1) for i, (b, s) in enumerate(ents)]


def _route(vertex, edges):
    eo = edges // EDGE_SH
    le = edges % EDGE_SH
    vo = vertex // NODE_SH
    lv = vertex % NODE_SH
    reg = (le >= REG_SPLIT).astype(np.int64)
    trow = np.where(reg == 0, eo * REG_ROWS[0] + le,
                    eo * REG_ROWS[1] + (le - REG_SPLIT))

    rA = _build_stream(eo, le // 512, vertex, le.astype(np.float32),
                       AW, TA_CAP)
    if rA is None:
        return None
    gaA, sgA, cntA = rA
    rC = _build_stream(vo, reg * NSC + lv // SCN, trow,
                       lv.astype(np.float32), 2 * NSC, TC_CAP)
    if rC is None:
        return None
    gaC, sgC, cntC = rC

    # Static counts rounded up to a multiple of 128: every gathered/loaded
    # block is fully populated (pad slots: idx 0 / segid -1, one-hot zeroes
    # them), so no stale-slot zero-fill is needed.
    cntA_max = np.minimum(-(-cntA.max(axis=0) // 128) * 128, TA_CAP)
    cntC_max = np.minimum(-(-cntC.max(axis=0) // 128) * 128, TC_CAP)

    # ---- stage A schedule: one cell per 512-edge window
    entA = []          # per window: [(b, base)] for the segmb image
    mmA = []           # per window: [(e0, b, s0, nsp, st, sp)]
    for w in range(AW):
        nsub = 4 if w < AW - 1 else 1
        ents = _cell_entries(sgA, w, cntA_max[w], w * 4, nsub)
        if ents is None:
            return None
        entA.append([(b, (w * 4 + s) * 128) for (b, s) in ents])
        runs = _runs(ents)
        mmA.append([(e0, b, s0, nsp, i == 0, i == len(runs) - 1)
                    for i, (e0, b, s0, nsp) in enumerate(runs)])

    # ---- stage C schedule: one cell per (region, supercell); mms grouped
    # by 512-node PSUM window with per-window start/stop flags
    entC = []
    mmC = []           # per cell: per window w_rel: [(e0, b, c0, nsp, st, sp)]
    for r in range(2):
        for sc in range(NSC):
            k = r * NSC + sc
            nsub = 20 if sc < NSC - 1 else 18
            ents = _cell_entries(sgC, k, cntC_max[k], sc * 20, nsub)
            if ents is None:
                return None
            entC.append([(b, (sc * 20 + s) * 128) for (b, s) in ents])
            runs = _runs(ents)
            wins = []
            for w_rel in range((nsub + 3) // 4):
                rw = [(e0, b, s0, nsp) for (e0, b, s0, nsp) in runs
                      if s0 // 4 == w_rel]
                if not rw:
                    return None
                wins.append([(e0, b, s0 % 4, nsp, i == 0, i == len(rw) - 1)
                             for i, (e0, b, s0, nsp) in enumerate(rw)])
            mmC.append(wins)

    emaxA = max(len(x) for x in entA)
    emaxC = max(len(x) for x in entC)
    offA = np.concatenate([[0], np.cumsum(cntA_max // 128)])
    offC16 = np.concatenate([[0], np.cumsum(cntC_max // 16)])
    sig = repr((tuple(cntA_max), tuple(cntC_max), mmA, mmC, emaxA, emaxC))
    return dict(gaA=gaA, cntA=cntA_max, mmA=mmA, entA=entA, offA=offA,
                gaC=gaC, sgC=sgC, cntC=cntC_max, mmC=mmC, entC=entC,
                sgA=sgA, offC16=offC16, emaxA=emaxA, emaxC=emaxC, sig=sig)


def _numpy_fallback(X, vertex, edges, w_b, w_a, w_c, b_b, b_a, b_c):
    Xe = np.zeros((E, F), np.float32)
    np.add.at(Xe, edges, X[vertex])
    Xv2 = np.zeros((N, F), np.float32)
    np.add.at(Xv2, vertex, Xe[edges])
    deg = np.bincount(vertex, minlength=N).astype(np.float32)[:, None]
    Xv = np.concatenate([deg * X, Xv2], axis=1)
    center = Xv @ w_b + b_b
    aXv = np.abs(Xv)
    return (center.astype(np.float32),
            (center - (aXv @ w_a + b_a)).astype(np.float32),
            (center + (aXv @ w_c + b_c)).astype(np.float32))


# ------------------------------------------------------------- bass program
def _build_program(meta):
    from concourse import bacc, tile
    import concourse.mybir as mybir

    f32 = mybir.dt.float32
    bf16 = mybir.dt.bfloat16
    i16 = mybir.dt.int16
    Alu = mybir.AluOpType
    Abs = mybir.ActivationFunctionType.Abs
    Copy = mybir.ActivationFunctionType.Copy
    Ident = mybir.ActivationFunctionType.Identity

    cntA, mmA, entA, offA = meta["cntA"], meta["mmA"], meta["entA"], meta["offA"]
    cntC, mmC, entC = meta["cntC"], meta["mmC"], meta["entC"]
    offC16 = meta["offC16"]
    emaxA, emaxC = meta["emaxA"], meta["emaxC"]
    emax = max(emaxA, emaxC)
    TOTA = int(offA[-1])
    COLC = int(offC16[-1])
    maxblkA = int((cntA // 128).max())
    maxblkC = int((cntC // 128).max())

    NQ = 4
    nc = bacc.Bacc(None, target_bir_lowering=False, debug=False,
                   num_devices=NC, num_swdge_queues=NQ)

    xva_d = nc.dram_tensor("xva", [128, TOTA * F], bf16, kind="ExternalInput")
    gaC_d = nc.dram_tensor("gaC", [128, COLC], i16, kind="ExternalInput")
    sgA_d = nc.dram_tensor("sgA", [128, AW * emaxA], f32, kind="ExternalInput")
    sgC_d = nc.dram_tensor("sgC", [128, 2 * NSC * emaxC], f32, kind="ExternalInput")
    xdt_d = nc.dram_tensor("xdt", [128, NODE_SH_P], bf16, kind="ExternalInput")
    axdt_d = nc.dram_tensor("axdt", [128, NODE_SH_P], bf16, kind="ExternalInput")
    wb1_d = nc.dram_tensor("wb1", [F, F], bf16, kind="ExternalInput")
    wb2_d = nc.dram_tensor("wb2", [F, F], bf16, kind="ExternalInput")
    wa1_d = nc.dram_tensor("wa1", [F, F], bf16, kind="ExternalInput")
    wa2_d = nc.dram_tensor("wa2", [F, F], bf16, kind="ExternalInput")
    wc1_d = nc.dram_tensor("wc1", [F, F], bf16, kind="ExternalInput")
    wc2_d = nc.dram_tensor("wc2", [F, F], bf16, kind="ExternalInput")
    bcols_d = nc.dram_tensor("bcols", [128, 3], f32, kind="ExternalInput")
    iota_d = nc.dram_tensor("iota", [128, emax * 128], f32, kind="ExternalInput")
    ident_d = nc.dram_tensor("ident", [128, 128], bf16, kind="ExternalInput")
    out3_d = nc.dram_tensor("out3T", [128, 3 * NODE_SH_P], bf16,
                            kind="ExternalOutput")

    xe = [nc.dram_tensor(f"xe{r}", [REG_ROWS[r], F], bf16) for r in range(2)]
    xt = [nc.dram_tensor(f"xt{r}", [TBL_ROWS[r], F], bf16, addr_space="Shared")
          for r in range(2)]

    with tile.TileContext(nc) as tc:
        with (
            tc.tile_pool(name="cp", bufs=1) as cp,
            tc.tile_pool(name="da", bufs=2) as da,
            tc.tile_pool(name="dc", bufs=2) as dc,
            tc.tile_pool(name="ohp", bufs=2) as ohp,
            tc.tile_pool(name="sp", bufs=2) as sp,
            tc.tile_pool(name="psw", bufs=2, space="PSUM") as psw,
            tc.tile_pool(name="psd", bufs=4, space="PSUM") as psd,
        ):
            # ---- constants / preloads
            iota = cp.tile([128, emax * 128], f32, tag="iota")
            nc.scalar.dma_start(iota[:], iota_d[:])
            identb = cp.tile([128, 128], bf16, tag="identb")
            nc.scalar.dma_start(identb[:], ident_d[:])
            ws = {}
            for nm, d in (("wb1", wb1_d), ("wb2", wb2_d), ("wa1", wa1_d),
                          ("wa2", wa2_d), ("wc1", wc1_d), ("wc2", wc2_d)):
                t = cp.tile([F, F], bf16, tag=nm, name=nm)
                nc.scalar.dma_start(t[:], d[:])
                ws[nm] = t
            bcols = cp.tile([128, 3], f32, tag="bcols")
            nc.scalar.dma_start(bcols[:], bcols_d[:])
            gaC_sb = cp.tile([128, COLC], i16, tag="gaC_sb")
            nc.scalar.dma_start(gaC_sb[:], gaC_d[:])
            sgA_sb = cp.tile([128, AW * emaxA], f32, tag="sgA_sb")
            nc.scalar.dma_start(sgA_sb[:], sgA_d[:])
            sgC_sb = cp.tile([128, 2 * NSC * emaxC], f32, tag="sgC_sb")
            nc.scalar.dma_start(sgC_sb[:], sgC_d[:])
            xv2sb = cp.tile([128, NODE_SH_P], bf16, tag="xv2sb")

            def build_oh(sg_sb, emax_s, k, nent, tag):
                # One DVE op builds every one-hot block of cell k:
                # oh[p, (e, j)] = (iota[j] == segmb[p, k*emax+e])
                oh = ohp.tile([128, emax * 128], bf16, tag=tag)
                segb = sg_sb[:, k * emax_s:k * emax_s + nent]
                nc.vector.tensor_tensor(
                    oh[:, :nent * 128].rearrange("p (e j) -> p e j", e=nent),
                    iota[:, :nent * 128].rearrange("p (e j) -> p e j", e=nent),
                    segb.unsqueeze(-1).broadcast_to([128, nent, 128]),
                    op=Alu.is_equal)
                return oh

            # ---- stage A: Xe^T windows from the host-gathered stream
            for w in range(AW):
                nsub = 4 if w < AW - 1 else 1
                wsz = nsub * 128
                nblk = int(cntA[w]) // 128
                dat = da.tile([128, maxblkA, F], bf16, tag="dA")
                nc.sync.dma_start(
                    dat[:, :nblk, :],
                    xva_d[:, int(offA[w]) * F:(int(offA[w]) + nblk) * F]
                    .rearrange("p (b f) -> p b f", b=nblk))
                oh = build_oh(sgA_sb, emaxA, w, len(entA[w]), "ohA")
                ps = psw.tile([128, 512], f32, tag="psA")
                for (e0, b, s0, nsp, st, sp_) in mmA[w]:
                    nc.tensor.matmul(ps[:, s0 * 128:(s0 + nsp) * 128],
                                     dat[:, b, :],
                                     oh[:, e0 * 128:(e0 + nsp) * 128],
                                     start=st, stop=sp_)
                xs = sp.tile([128, 512], bf16, tag="xs")
                nc.scalar.activation(xs[:, :wsz], ps[:, :wsz], Copy)
                pt = psw.tile([128, 512], bf16, tag="psA")
                for s in range(nsub):
                    nc.tensor.matmul(pt[:, s * 128:(s + 1) * 128],
                                     xs[:, s * 128:(s + 1) * 128], identb[:],
                                     is_transpose=True,
                                     start=(s == 0), stop=(s == nsub - 1))
                xo = sp.tile([128, 4, 128], bf16, tag="xo")
                nc.vector.tensor_copy(
                    xo[:, :nsub, :],
                    pt[:, :wsz].rearrange("p (s j) -> p s j", s=nsub))
                r, row = (0, w * 512) if w < 6 else (1, w * 512 - 3072)
                nc.scalar.dma_start(
                    xe[r][row:row + wsz, :].rearrange("(s p) j -> p s j", p=128),
                    xo[:, :nsub, :])
                if w == 5:
                    nc.gpsimd.collective_compute(
                        "AllGather", Alu.bypass,
                        replica_groups=[list(range(NC))],
                        ins=[xe[0].ap().opt()], outs=[xt[0].ap().opt()])
                if w == AW - 1:
                    nc.gpsimd.collective_compute(
                        "AllGather", Alu.bypass,
                        replica_groups=[list(range(NC))],
                        ins=[xe[1].ap().opt()], outs=[xt[1].ap().opt()])

            # ---- stage C: Xv2^T via supercell gathers + fused dense head
            for r in range(2):
                for sc in range(NSC):
                    k = r * NSC + sc
                    Tk = int(cntC[k])
                    dat = dc.tile([128, maxblkC, F], bf16, tag="dC")
                    co = int(offC16[k])
                    for j0 in range(0, Tk, GCALL):
                        n = min(GCALL, Tk - j0)
                        nc.gpsimd.dma_gather(
                            dat[:, j0 // 128:(j0 + n) // 128, :], xt[r].ap(),
                            gaC_sb[:, co + j0 // 16:co + (j0 + n) // 16],
                            n, n, F)
                    oh = build_oh(sgC_sb, emaxC, k, len(entC[k]), "ohC")
                    nwin = len(mmC[k])
                    for w_rel in range(nwin):
                        w = sc * 5 + w_rel
                        wsz = 512 if w < CW - 1 else 256
                        ps = psw.tile([128, 512], f32, tag="psC")
                        for (e0, b, c0, nsp, st, sp_) in mmC[k][w_rel]:
                            nc.tensor.matmul(
                                ps[:, c0 * 128:(c0 + nsp) * 128],
                                dat[:, b, :],
                                oh[:, e0 * 128:(e0 + nsp) * 128],
                                start=st, stop=sp_)
                        sl = xv2sb[:, w * 512:w * 512 + wsz]
                        if r == 0:
                            nc.vector.tensor_copy(sl, ps[:, :wsz])
                            continue
                        nc.vector.tensor_add(sl, sl, ps[:, :wsz])

                        # fused dense head on this 512-node window (^T form)
                        base = w * 512
                        xdw = sp.tile([128, 512], bf16, tag="xdw")
                        nc.sync.dma_start(xdw[:, :wsz],
                                          xdt_d[:, base:base + wsz])
                        axw = sp.tile([128, 512], bf16, tag="axw")
                        nc.sync.dma_start(axw[:, :wsz],
                                          axdt_d[:, base:base + wsz])
                        av2 = sp.tile([128, 512], bf16, tag="av2")
                        nc.scalar.activation(av2[:, :wsz], sl, Abs)
                        pc = psd.tile([128, 512], f32, tag="hd")
                        nc.tensor.matmul(pc[:, :wsz], ws["wb1"][:],
                                         xdw[:, :wsz], start=True, stop=False)
                        nc.tensor.matmul(pc[:, :wsz], ws["wb2"][:],
                                         sl, start=False, stop=True)
                        pa = psd.tile([128, 512], f32, tag="hd")
                        nc.tensor.matmul(pa[:, :wsz], ws["wa1"][:],
                                         axw[:, :wsz], start=True, stop=False)
                        nc.tensor.matmul(pa[:, :wsz], ws["wa2"][:],
                                         av2[:, :wsz], start=False, stop=True)
                        pcr = psd.tile([128, 512], f32, tag="hd")
                        nc.tensor.matmul(pcr[:, :wsz], ws["wc1"][:],
                                         axw[:, :wsz], start=True, stop=False)
                        nc.tensor.matmul(pcr[:, :wsz], ws["wc2"][:],
                                         av2[:, :wsz], start=False, stop=True)
                        otc = sp.tile([128, 512], bf16, tag="otc")
                        nc.scalar.activation(otc[:, :wsz], pc[:, :wsz], Ident,
                                             bias=bcols[:, 0:1])
                        paB = sp.tile([128, 512], bf16, tag="paB")
                        nc.scalar.activation(paB[:, :wsz], pa[:, :wsz], Ident,
                                             bias=bcols[:, 1:2])
                        pcrB = sp.tile([128, 512], bf16, tag="pcrB")
                        nc.scalar.activation(pcrB[:, :wsz], pcr[:, :wsz],
                                             Ident, bias=bcols[:, 2:3])
                        hl = sp.tile([128, 512], bf16, tag="hl")
                        nc.vector.tensor_tensor(hl[:, :wsz], otc[:, :wsz],
                                                paB[:, :wsz], op=Alu.subtract)
                        hr = sp.tile([128, 512], bf16, tag="hr")
                        nc.vector.tensor_tensor(hr[:, :wsz], otc[:, :wsz],
                                                pcrB[:, :wsz], op=Alu.add)
                        # out writes stay off gpsimd: Pool-engine DMA insts
                        # consume DMASW semaphore lanes and would break the
                        # gather queue<->lane locking
                        nc.scalar.dma_start(
                            out3_d[:, base:base + wsz], otc[:, :wsz])
                        nc.sync.dma_start(
                            out3_d[:, NODE_SH_P + base:NODE_SH_P + base + wsz],
                            hl[:, :wsz])
                        nc.sync.dma_start(
                            out3_d[:, 2 * NODE_SH_P + base:
                                   2 * NODE_SH_P + base + wsz],
                            hr[:, :wsz])

    # SWDGE queue assignment must match the DMASW semaphore lane the Tile
    # sem-assignment pass gave each gather (lanes rotate over Pool-engine
    # DMA insts in SCHEDULED order; a lane's semaphore is locked to one
    # queue).  queue = lane % NQ spreads descriptor-gen across workers.
    from concourse.tile_sem_assignment import PROC_NAME_TO_IDX
    idx2lane = {PROC_NAME_TO_IDX[f"DMASW{i}"]: i for i in range(8)}
    for insts in tc.ordered_instructions_by_block.values():
        for inst in insts:
            if isinstance(inst, mybir.InstDMAGatherAnt):
                lane = idx2lane.get(getattr(inst, "bass_scheduled_proc", -1))
                if lane is not None:
                    inst.queue_num = lane % NQ

    nc.compile()
    return nc


# ------------------------------------------------------------------- driver
def kernel(X, vertex, edges, X0, n_edges, w_b, w_a, w_c, b_b, b_a, b_c):
    from concourse.bass_utils import run_bass_kernel_spmd
    import ml_dtypes

    bf = ml_dtypes.bfloat16
    X = np.ascontiguousarray(np.asarray(X, dtype=np.float32))
    vertex = np.asarray(vertex).astype(np.int64)
    edges = np.asarray(edges).astype(np.int64)
    w_b = np.asarray(w_b, dtype=np.float32)
    w_a = np.asarray(w_a, dtype=np.float32)
    w_c = np.asarray(w_c, dtype=np.float32)
    b_b = np.asarray(b_b, dtype=np.float32).reshape(1, F)
    b_a = np.asarray(b_a, dtype=np.float32).reshape(1, F)
    b_c = np.asarray(b_c, dtype=np.float32).reshape(1, F)

    meta = _route(vertex, edges)
    if meta is None:
        return _numpy_fallback(X, vertex, edges, w_b, w_a, w_c, b_b, b_a, b_c)

    if _STATE.get("sig") != meta["sig"]:
        _STATE["nc"] = _build_program(meta)
        _STATE["sig"] = meta["sig"]
    nc = _STATE["nc"]

    Xbf = X.astype(bf)
    deg = np.bincount(vertex, minlength=N).astype(np.float32)
    XD = (X * deg[:, None]).astype(np.float32)

    emax = max(meta["emaxA"], meta["emaxC"])
    iota_np = np.ascontiguousarray(
        np.tile(np.arange(128, dtype=np.float32), (128, emax)))
    ident_np = np.ascontiguousarray(np.eye(128, dtype=np.float32).astype(bf))
    wcasts = {nm: np.ascontiguousarray(w.astype(bf))
              for nm, w in (("wb1", w_b[:F]), ("wb2", w_b[F:]),
                            ("wa1", w_a[:F]), ("wa2", w_a[F:]),
                            ("wc1", w_c[:F]), ("wc2", w_c[F:]))}
    bcols_np = np.ascontiguousarray(
        np.stack([b_b.ravel(), b_a.ravel(), b_c.ravel()], axis=1)
        .astype(np.float32))

    cntA, offA = meta["cntA"], meta["offA"]
    TOTA = int(offA[-1])
    in_maps = []
    for m in range(NC):
        # host-gathered stage A token stream in the [128, block, F] layout
        xva = np.empty((128, TOTA, F), bf)
        for w in range(AW):
            Tw = int(cntA[w])
            rows = Xbf[meta["gaA"][m, w, :Tw]]          # [Tw, F]
            xva[:, int(offA[w]):int(offA[w]) + Tw // 128, :] = (
                rows.reshape(Tw // 128, 128, F).transpose(1, 0, 2))
        xdm = np.zeros((128, NODE_SH_P), np.float32)
        xdm[:, :NODE_SH] = XD[m * NODE_SH:(m + 1) * NODE_SH].T
        im = {
            "xva": np.ascontiguousarray(xva.reshape(128, TOTA * F)),
            "gaC": _wrap16v([meta["gaC"][m, k, :int(meta["cntC"][k])]
                             for k in range(2 * NSC)]),
            "sgA": _segmb_img(meta["sgA"][m], meta["entA"], meta["emaxA"]),
            "sgC": _segmb_img(meta["sgC"][m], meta["entC"], meta["emaxC"]),
            "xdt": np.ascontiguousarray(xdm.astype(bf)),
            "axdt": np.ascontiguousarray(np.abs(xdm).astype(bf)),
            "bcols": bcols_np,
            "iota": iota_np, "ident": ident_np,
        }
        im.update(wcasts)
        in_maps.append(im)

    res = run_bass_kernel_spmd(nc, in_maps, list(range(NC)))
    center = np.empty((N, F), np.float32)
    hl = np.empty((N, F), np.float32)
    hr = np.empty((N, F), np.float32)
    for m in range(NC):
        o = np.asarray(res.results[m]["out3T"]).astype(np.float32)
        sl = slice(m * NODE_SH, (m + 1) * NODE_SH)
        center[sl] = o[:, :NODE_SH].T
        hl[sl] = o[:, NODE_SH_P:NODE_SH_P + NODE_SH].T
        hr[sl] = o[:, 2 * NODE_SH_P:2 * NODE_SH_P + NODE_SH].T
    return center, hl, hr


# revision 24
# speedup vs baseline: 1.5919x; 1.0011x over previous
"""Trainium2 Bass kernel v3 for nn_CrispToFuzzyConv (hypergraph message passing).

Math (see reference):
  Xe   = segment_sum(X[vertex], edges, E)                 # round 1
  Xv   = concat([deg * X, Xv2]),  Xv2 = segment_sum(Xe[edges], vertex, N)
  center = Xv @ w_b + b_b
  HL = center - (|Xv| @ w_a + b_a)
  HR = center + (|Xv| @ w_c + b_c)

v3 strategy (vs v2):
  - Stage A consumes a HOST-PRE-GATHERED token stream (X[vertex] rows sorted
    by (edge window, edge id), already in the [128, block, F] PE layout).
    This removes all 52 stage-A SWDGE gather calls (the v2 trace showed the
    gpsimd/Pool engine 65% busy, ~3.5us per dma_gather call) and the 4-chunk
    cell split; stage A becomes plain sequential DMA + one-hot matmuls.
  - Stage C keeps device gathers (Xe is device-computed) but decouples the
    gather-call granularity from the PSUM window: one call per (region,
    supercell of 5 node windows) = 10 calls total instead of 50.  mm entries
    are emitted grouped by PSUM window so only one window accumulates at a
    time per ring slot.
  - AllGather tables are addr_space="Shared" (fast HBM-HBM collective path;
    measured cc_op 225us -> 192us on v2).
  - Dense head runs transposed: out^T[f_out, node] = w^T-stationary matmuls
    with 512-node moving operands (4x fewer, 4x larger PE instructions than
    v2's per-128-node form), biases folded into the PSUM->SBUF copy via
    scalar Identity activation with a per-partition bias column, |deg*X|
    precomputed on host.  The host transposes the [128, 3*nodes] result.
  - PSUM: psA ring 2 banks (shared with the transpose tiles), psC ring 2,
    head ring 4 (3 tiles/window -> 1.33 windows in flight) = 8 banks.

Hardware constraints baked in (from v1/v2 sessions):
  - dma_gather idx dtype is int16 -> gather tables <= 32767 rows: Xe tables
    split in 2 regions (24576/25600 rows)
  - elem_size_bytes % 256 == 0 -> bf16 F=128 rows (256B) are legal
  - gather output layout: token i -> partition i%128, col-block i//128
  - SWDGE requires num_idxs_reg == #(idx >= 0); every slot of a call is kept
    valid (counts rounded up to 128 with idx-0/segid-(-1) pad slots) so all
    gathered blocks are fully DMA-written
  - PSUM "zero regions" are 2KB banks; every PSUM tile is padded to a bank
  - the Tile sem-assignment pass locks each DMASW semaphore lane to one
    SWDGE queue; queue_num is assigned AFTER scheduling as lane % 4
"""

import numpy as np

# ---------------------------------------------------------------- constants
N = 100000
E = 50000
NNZ = 300000
F = 128
NC = 8

NODE_SH = 12500              # nodes per core
NODE_SH_P = 12544            # 98 subs of 128
CW = 25                      # 512-node windows (last = 256)
NSC = 5                      # supercells per region (5 windows each)
SCN = 2560                   # nodes per supercell
TC_CAP = 6144                # slot cap per stage-C gather call

EDGE_SH = 6250               # edges per core
AW = 13                      # 512-edge windows (last = 128)
NSUBA = 49                   # 128-edge subs per core

REG_SPLIT = 3072             # local edge rows [0,3072) -> region 0
REG_ROWS = (3072, 3200)      # padded local rows per region
TBL_ROWS = (NC * 3072, NC * 3200)

GCALL = 1024                 # max slots per dma_gather sub-call (ucode limit:
                             # 2048 hangs the device, 1024 is proven)

_STATE = {}


# ---------------------------------------------------------------- host side
def _wrap16v(idx_cells):
    """list of per-cell [T_k] int arrays -> [128, sum(T_k/16)] int16 image
    (idx i at partition i%16, col co_k + i//16; replicated across the 8
    groups of 16)."""
    cols = sum(a.shape[0] // 16 for a in idx_cells)
    img = np.zeros((128, cols), np.int16)
    co = 0
    for a in idx_cells:
        t = a.reshape(-1, 16).T.astype(np.int16)      # [16, T/16]
        img[:, co:co + t.shape[1]] = np.tile(t, (8, 1))
        co += t.shape[1]
    return np.ascontiguousarray(img)


def _segmb_img(sg, entries, emax):
    """Per-entry biased segid image: [128, ncell*emax] fp16 where column
    k*emax+e holds segid(block(e)) - base(e) for cell k, a small integer in
    [0, 128) (fp16-exact).  Pad slots (segid -1) and unused entry columns
    (-30000) never match the one-hot iota."""
    ncell = len(entries)
    img = np.full((128, ncell * emax), -30000.0, np.float16)
    for k in range(ncell):
        for e, (b, base) in enumerate(entries[k]):
            img[:, k * emax + e] = (sg[k, b * 128:(b + 1) * 128] - base
                                    ).astype(np.float16)
    return np.ascontiguousarray(img)


def _build_stream(owner, cell, gidx, segid, ncells, T):
    """Pack tokens into per-(core, cell) sorted slot arrays."""
    ga = np.zeros((NC, ncells, T), np.int64)
    sg = np.full((NC, ncells, T), -1.0, np.float32)
    cnt = np.zeros((NC, ncells), np.int64)
    for m in range(NC):
        s = np.nonzero(owner == m)[0]
        o = np.lexsort((segid[s], cell[s]))
        s = s[o]
        cs = cell[s]
        bounds = np.searchsorted(cs, np.arange(ncells + 1))
        for k in range(ncells):
            lo, hi = int(bounds[k]), int(bounds[k + 1])
            n = hi - lo
            if n > T:
                return None
            ga[m, k, :n] = gidx[s[lo:hi]]
            sg[m, k, :n] = segid[s[lo:hi]]
            cnt[m, k] = n
    return ga, sg, cnt


def _cell_entries(sg_all, k, cnt_max, sub_base, nsub):
    """Static (block, sub) entry list for one cell, from the union of all
    cores' segids.  subs relative to sub_base; None if out of range or if
    any sub in [0, nsub) has no tokens (its PSUM cols would stay unwritten).
    """
    entries = []
    subs_seen = set()
    nb = -(-int(cnt_max) // 128)
    for b in range(nb):
        vals = sg_all[:, k, b * 128:(b + 1) * 128].ravel()
        vals = vals[vals >= 0].astype(np.int64)
        if len(vals) == 0:
            continue
        for sub in np.unique(vals // 128):
            s = int(sub) - sub_base
            if s < 0 or s >= nsub:
                return None
            entries.append((b, s))
            subs_seen.add(s)
    if subs_seen != set(range(nsub)):
        return None
    return entries


def _runs(ents):
    """One matmul per (block, sub) entry.  Runs are NOT merged across sub
    boundaries: the PSUM zero-region is the whole 2KB bank, so an mm view
    must be uniformly pending-zero (first touch of a sub after the group's
    single start_tensor_calc) or uniformly written (accumulation) -- a
    merged multi-sub view would mix the two states."""
    return [(i, b, s, 1) for i, (b, s) in enumerate(ents)]


def _route(vertex, edges):
    eo = edges // EDGE_SH
    le = edges % EDGE_SH
    vo = vertex // NODE_SH
    lv = vertex % NODE_SH
    reg = (le >= REG_SPLIT).astype(np.int64)
    trow = np.where(reg == 0, eo * REG_ROWS[0] + le,
                    eo * REG_ROWS[1] + (le - REG_SPLIT))

    # ---- stage A: static per-sub slot layout.  Sub s (128 edges) gets
    # blkA[s] = ceil(max-over-cores count / 128) full blocks, so every
    # 128-token block belongs to exactly one sub: entries == blocks, no
    # boundary crossings, and the relative-segid image is just the slot
    # array reshaped.  Pad slots: gidx 0 / segid -1.
    subA = le // 128
    cntAms = np.zeros((NC, NSUBA), np.int64)
    np.add.at(cntAms, (eo, subA), 1)
    blkA = np.maximum(1, -(-cntAms.max(axis=0) // 128))
    soffA = np.concatenate([[0], np.cumsum(blkA)]).astype(np.int64)
    TOTA = int(soffA[-1])
    gaA = np.zeros((NC, TOTA * 128), np.int64)
    sgA = np.full((NC, TOTA * 128), -1.0, np.float32)
    for m in range(NC):
        sel = np.nonzero(eo == m)[0]
        sel = sel[np.argsort(subA[sel], kind="stable")]
        ss = subA[sel]
        bounds = np.searchsorted(ss, np.arange(NSUBA + 1))
        for s in range(NSUBA):
            lo, hi = int(bounds[s]), int(bounds[s + 1])
            base = int(soffA[s]) * 128
            gaA[m, base:base + hi - lo] = vertex[sel[lo:hi]]
            sgA[m, base:base + hi - lo] = le[sel[lo:hi]] - s * 128

    rC = _build_stream(vo, reg * NSC + lv // SCN, trow,
                       lv.astype(np.float32), 2 * NSC, TC_CAP)
    if rC is None:
        return None
    gaC, sgC, cntC = rC
    cntC_max = np.minimum(-(-cntC.max(axis=0) // 128) * 128, TC_CAP)

    # stage A mm schedule: per window, one mm per (block, its sub)
    mmA = []           # per window: [(e_local, s_local, st, sp)]
    for w in range(AW):
        slo, shi = 4 * w, min(4 * w + 4, NSUBA)
        ents = [(int(e - soffA[slo]), s - slo)
                for s in range(slo, shi)
                for e in range(int(soffA[s]), int(soffA[s + 1]))]
        mmA.append([(e, s, i == 0, i == len(ents) - 1)
                    for i, (e, s) in enumerate(ents)])

    # ---- stage C schedule: one cell per (region, supercell); mms grouped
    # by 512-node PSUM window with per-window start/stop flags
    entC = []
    mmC = []           # per cell: per window w_rel: [(e0, b, c0, nsp, st, sp)]
    for r in range(2):
        for sc in range(NSC):
            k = r * NSC + sc
            nsub = 20 if sc < NSC - 1 else 18
            ents = _cell_entries(sgC, k, cntC_max[k], sc * 20, nsub)
            if ents is None:
                return None
            entC.append([(b, (sc * 20 + s) * 128) for (b, s) in ents])
            runs = _runs(ents)
            wins = []
            for w_rel in range((nsub + 3) // 4):
                rw = [(e0, b, s0, nsp) for (e0, b, s0, nsp) in runs
                      if s0 // 4 == w_rel]
                if not rw:
                    return None
                wins.append([(e0, b, s0 % 4, nsp, i == 0, i == len(rw) - 1)
                             for i, (e0, b, s0, nsp) in enumerate(rw)])
            mmC.append(wins)

    emaxC = max(len(x) for x in entC)
    offC16 = np.concatenate([[0], np.cumsum(cntC_max // 16)])
    sig = repr((tuple(blkA), tuple(cntC_max), mmA, mmC, emaxC))
    return dict(gaA=gaA, sgA=sgA, blkA=blkA, soffA=soffA, mmA=mmA,
                gaC=gaC, sgC=sgC, cntC=cntC_max, mmC=mmC, entC=entC,
                offC16=offC16, emaxC=emaxC, sig=sig)


def _numpy_fallback(X, vertex, edges, w_b, w_a, w_c, b_b, b_a, b_c):
    Xe = np.zeros((E, F), np.float32)
    np.add.at(Xe, edges, X[vertex])
    Xv2 = np.zeros((N, F), np.float32)
    np.add.at(Xv2, vertex, Xe[edges])
    deg = np.bincount(vertex, minlength=N).astype(np.float32)[:, None]
    Xv = np.concatenate([deg * X, Xv2], axis=1)
    center = Xv @ w_b + b_b
    aXv = np.abs(Xv)
    return (center.astype(np.float32),
            (center - (aXv @ w_a + b_a)).astype(np.float32),
            (center + (aXv @ w_c + b_c)).astype(np.float32))


# ------------------------------------------------------------- bass program
def _build_program(meta):
    from concourse import bacc, tile
    import concourse.mybir as mybir

    f32 = mybir.dt.float32
    bf16 = mybir.dt.bfloat16
    f16 = mybir.dt.float16
    i16 = mybir.dt.int16
    Alu = mybir.AluOpType
    Abs = mybir.ActivationFunctionType.Abs
    Copy = mybir.ActivationFunctionType.Copy
    Ident = mybir.ActivationFunctionType.Identity

    mmA, blkA, soffA = meta["mmA"], meta["blkA"], meta["soffA"]
    cntC, mmC, entC = meta["cntC"], meta["mmC"], meta["entC"]
    offC16 = meta["offC16"]
    emaxC = meta["emaxC"]
    TOTA = int(soffA[-1])
    COLC = int(offC16[-1])
    maxblkA = int(max(soffA[min(4 * w + 4, NSUBA)] - soffA[4 * w]
                      for w in range(AW)))
    maxblkC = int((cntC // 128).max())
    emax = max(maxblkA, emaxC)

    NQ = 4
    nc = bacc.Bacc(None, target_bir_lowering=False, debug=False,
                   num_devices=NC, num_swdge_queues=NQ)

    # small inputs declared (and so uploaded) first: the kernel's prologue
    # only needs these, and the big xva stream can trickle in behind it
    gaC_d = nc.dram_tensor("gaC", [128, COLC], i16, kind="ExternalInput")
    sgA_d = nc.dram_tensor("sgA", [128, TOTA], f16, kind="ExternalInput")
    sgC_d = nc.dram_tensor("sgC", [128, 2 * NSC * emaxC], f16, kind="ExternalInput")
    xdt_d = nc.dram_tensor("xdt", [128, NODE_SH_P], bf16, kind="ExternalInput")
    axdt_d = nc.dram_tensor("axdt", [128, NODE_SH_P], bf16, kind="ExternalInput")
    wb1_d = nc.dram_tensor("wb1", [F, F], bf16, kind="ExternalInput")
    wb2_d = nc.dram_tensor("wb2", [F, F], bf16, kind="ExternalInput")
    wa1_d = nc.dram_tensor("wa1", [F, F], bf16, kind="ExternalInput")
    wa2_d = nc.dram_tensor("wa2", [F, F], bf16, kind="ExternalInput")
    wc1_d = nc.dram_tensor("wc1", [F, F], bf16, kind="ExternalInput")
    wc2_d = nc.dram_tensor("wc2", [F, F], bf16, kind="ExternalInput")
    bcols_d = nc.dram_tensor("bcols", [128, 3], f32, kind="ExternalInput")
    iota_d = nc.dram_tensor("iota", [128, emax * 128], f16, kind="ExternalInput")
    ident_d = nc.dram_tensor("ident", [128, 128], bf16, kind="ExternalInput")
    xva_d = nc.dram_tensor("xva", [128, TOTA * F], bf16, kind="ExternalInput")
    out3_d = nc.dram_tensor("out3T", [128, 3 * NODE_SH_P], bf16,
                            kind="ExternalOutput")

    xe = [nc.dram_tensor(f"xe{r}", [REG_ROWS[r], F], bf16) for r in range(2)]
    xt = [nc.dram_tensor(f"xt{r}", [TBL_ROWS[r], F], bf16, addr_space="Shared")
          for r in range(2)]

    with tile.TileContext(nc) as tc:
        with (
            tc.tile_pool(name="cp", bufs=1) as cp,
            tc.tile_pool(name="da", bufs=2) as da,
            tc.tile_pool(name="dc", bufs=2) as dc,
            tc.tile_pool(name="ohp", bufs=2) as ohp,
            tc.tile_pool(name="sp", bufs=2) as sp,
            tc.tile_pool(name="psw", bufs=2, space="PSUM") as psw,
            tc.tile_pool(name="psd", bufs=4, space="PSUM") as psd,
        ):
            # ---- constants / preloads
            iota = cp.tile([128, emax * 128], f16, tag="iota")
            nc.scalar.dma_start(iota[:], iota_d[:])
            identb = cp.tile([128, 128], bf16, tag="identb")
            nc.scalar.dma_start(identb[:], ident_d[:])
            ws = {}
            for nm, d in (("wb1", wb1_d), ("wb2", wb2_d), ("wa1", wa1_d),
                          ("wa2", wa2_d), ("wc1", wc1_d), ("wc2", wc2_d)):
                t = cp.tile([F, F], bf16, tag=nm, name=nm)
                nc.scalar.dma_start(t[:], d[:])
                ws[nm] = t
            bcols = cp.tile([128, 3], f32, tag="bcols")
            nc.scalar.dma_start(bcols[:], bcols_d[:])
            gaC_sb = cp.tile([128, COLC], i16, tag="gaC_sb")
            nc.scalar.dma_start(gaC_sb[:], gaC_d[:])
            sgA_sb = cp.tile([128, TOTA], f16, tag="sgA_sb")
            nc.scalar.dma_start(sgA_sb[:], sgA_d[:])
            sgC_sb = cp.tile([128, 2 * NSC * emaxC], f16, tag="sgC_sb")
            nc.scalar.dma_start(sgC_sb[:], sgC_d[:])
            xv2sb = cp.tile([128, NODE_SH_P], bf16, tag="xv2sb")

            def build_oh(segb, nent, tag):
                # One DVE op builds every one-hot block of a cell:
                # oh[p, (e, j)] = (iota[j] == segb[p, e]); fp16 operands run
                # the DVE in 16-bit (2x) mode
                oh = ohp.tile([128, emax * 128], bf16, tag=tag)
                nc.vector.tensor_tensor(
                    oh[:, :nent * 128].rearrange("p (e j) -> p e j", e=nent),
                    iota[:, :nent * 128].rearrange("p (e j) -> p e j", e=nent),
                    segb.unsqueeze(-1).broadcast_to([128, nent, 128]),
                    op=Alu.is_equal)
                return oh

            # ---- stage A: Xe^T windows from the host-gathered stream
            for w in range(AW):
                nsub = 4 if w < AW - 1 else 1
                wsz = nsub * 128
                b0 = int(soffA[4 * w])
                nblk = int(soffA[min(4 * w + 4, NSUBA)]) - b0
                dat = da.tile([128, maxblkA, F], bf16, tag="dA")
                nc.sync.dma_start(
                    dat[:, :nblk, :],
                    xva_d[:, b0 * F:(b0 + nblk) * F]
                    .rearrange("p (b f) -> p b f", b=nblk))
                oh = build_oh(sgA_sb[:, b0:b0 + nblk], nblk, "ohA")
                ps = psw.tile([128, 512], f32, tag="psA")
                for (e, s, st, sp_) in mmA[w]:
                    nc.tensor.matmul(ps[:, s * 128:(s + 1) * 128],
                                     dat[:, e, :],
                                     oh[:, e * 128:(e + 1) * 128],
                                     start=st, stop=sp_)
                xs = sp.tile([128, 512], bf16, tag="xs")
                nc.scalar.activation(xs[:, :wsz], ps[:, :wsz], Copy)
                pt = psw.tile([128, 512], bf16, tag="psA")
                for s in range(nsub):
                    nc.tensor.matmul(pt[:, s * 128:(s + 1) * 128],
                                     xs[:, s * 128:(s + 1) * 128], identb[:],
                                     is_transpose=True,
                                     start=(s == 0), stop=(s == nsub - 1))
                xo = sp.tile([128, 4, 128], bf16, tag="xo")
                nc.vector.tensor_copy(
                    xo[:, :nsub, :],
                    pt[:, :wsz].rearrange("p (s j) -> p s j", s=nsub))
                r, row = (0, w * 512) if w < 6 else (1, w * 512 - 3072)
                nc.scalar.dma_start(
                    xe[r][row:row + wsz, :].rearrange("(s p) j -> p s j", p=128),
                    xo[:, :nsub, :])
                if w == 5:
                    nc.gpsimd.collective_compute(
                        "AllGather", Alu.bypass,
                        replica_groups=[list(range(NC))],
                        ins=[xe[0].ap().opt()], outs=[xt[0].ap().opt()])
                if w == AW - 1:
                    nc.gpsimd.collective_compute(
                        "AllGather", Alu.bypass,
                        replica_groups=[list(range(NC))],
                        ins=[xe[1].ap().opt()], outs=[xt[1].ap().opt()])

            # ---- stage C: Xv2^T via supercell gathers + fused dense head
            for r in range(2):
                for sc in range(NSC):
                    k = r * NSC + sc
                    Tk = int(cntC[k])
                    dat = dc.tile([128, maxblkC, F], bf16, tag="dC")
                    co = int(offC16[k])
                    for j0 in range(0, Tk, GCALL):
                        n = min(GCALL, Tk - j0)
                        nc.gpsimd.dma_gather(
                            dat[:, j0 // 128:(j0 + n) // 128, :], xt[r].ap(),
                            gaC_sb[:, co + j0 // 16:co + (j0 + n) // 16],
                            n, n, F)
                    oh = build_oh(
                        sgC_sb[:, k * emaxC:k * emaxC + len(entC[k])],
                        len(entC[k]), "ohC")
                    nwin = len(mmC[k])
                    for w_rel in range(nwin):
                        w = sc * 5 + w_rel
                        wsz = 512 if w < CW - 1 else 256
                        ps = psw.tile([128, 512], f32, tag="psC")
                        for (e0, b, c0, nsp, st, sp_) in mmC[k][w_rel]:
                            nc.tensor.matmul(
                                ps[:, c0 * 128:(c0 + nsp) * 128],
                                dat[:, b, :],
                                oh[:, e0 * 128:(e0 + nsp) * 128],
                                start=st, stop=sp_)
                        sl = xv2sb[:, w * 512:w * 512 + wsz]
                        if r == 0:
                            # scalar engine is idle during the r0 sweep
                            nc.scalar.activation(sl, ps[:, :wsz], Copy)
                            continue
                        nc.vector.tensor_add(sl, sl, ps[:, :wsz])

                        # fused dense head on this 512-node window (^T form)
                        base = w * 512
                        xdw = sp.tile([128, 512], bf16, tag="xdw")
                        nc.sync.dma_start(xdw[:, :wsz],
                                          xdt_d[:, base:base + wsz])
                        axw = sp.tile([128, 512], bf16, tag="axw")
                        nc.sync.dma_start(axw[:, :wsz],
                                          axdt_d[:, base:base + wsz])
                        av2 = sp.tile([128, 512], bf16, tag="av2")
                        nc.scalar.activation(av2[:, :wsz], sl, Abs)
                        pc = psd.tile([128, 512], f32, tag="hd")
                        nc.tensor.matmul(pc[:, :wsz], ws["wb1"][:],
                                         xdw[:, :wsz], start=True, stop=False)
                        nc.tensor.matmul(pc[:, :wsz], ws["wb2"][:],
                                         sl, start=False, stop=True)
                        pa = psd.tile([128, 512], f32, tag="hd")
                        nc.tensor.matmul(pa[:, :wsz], ws["wa1"][:],
                                         axw[:, :wsz], start=True, stop=False)
                        nc.tensor.matmul(pa[:, :wsz], ws["wa2"][:],
                                         av2[:, :wsz], start=False, stop=True)
                        pcr = psd.tile([128, 512], f32, tag="hd")
                        nc.tensor.matmul(pcr[:, :wsz], ws["wc1"][:],
                                         axw[:, :wsz], start=True, stop=False)
                        nc.tensor.matmul(pcr[:, :wsz], ws["wc2"][:],
                                         av2[:, :wsz], start=False, stop=True)
                        otc = sp.tile([128, 512], bf16, tag="otc")
                        nc.scalar.activation(otc[:, :wsz], pc[:, :wsz], Ident,
                                             bias=bcols[:, 0:1])
                        paB = sp.tile([128, 512], bf16, tag="paB")
                        nc.scalar.activation(paB[:, :wsz], pa[:, :wsz], Ident,
                                             bias=bcols[:, 1:2])
                        pcrB = sp.tile([128, 512], bf16, tag="pcrB")
                        nc.scalar.activation(pcrB[:, :wsz], pcr[:, :wsz],
                                             Ident, bias=bcols[:, 2:3])
                        hl = sp.tile([128, 512], bf16, tag="hl")
                        nc.vector.tensor_tensor(hl[:, :wsz], otc[:, :wsz],
                                                paB[:, :wsz], op=Alu.subtract)
                        hr = sp.tile([128, 512], bf16, tag="hr")
                        nc.vector.tensor_tensor(hr[:, :wsz], otc[:, :wsz],
                                                pcrB[:, :wsz], op=Alu.add)
                        # out writes stay off gpsimd: Pool-engine DMA insts
                        # consume DMASW semaphore lanes and would break the
                        # gather queue<->lane locking
                        nc.scalar.dma_start(
                            out3_d[:, base:base + wsz], otc[:, :wsz])
                        nc.sync.dma_start(
                            out3_d[:, NODE_SH_P + base:NODE_SH_P + base + wsz],
                            hl[:, :wsz])
                        nc.sync.dma_start(
                            out3_d[:, 2 * NODE_SH_P + base:
                                   2 * NODE_SH_P + base + wsz],
                            hr[:, :wsz])

    # SWDGE queue assignment must match the DMASW semaphore lane the Tile
    # sem-assignment pass gave each gather (lanes rotate over Pool-engine
    # DMA insts in SCHEDULED order; a lane's semaphore is locked to one
    # queue).  queue = lane % NQ spreads descriptor-gen across workers.
    from concourse.tile_sem_assignment import PROC_NAME_TO_IDX
    idx2lane = {PROC_NAME_TO_IDX[f"DMASW{i}"]: i for i in range(8)}
    for insts in tc.ordered_instructions_by_block.values():
        for inst in insts:
            if isinstance(inst, mybir.InstDMAGatherAnt):
                lane = idx2lane.get(getattr(inst, "bass_scheduled_proc", -1))
                if lane is not None:
                    inst.queue_num = lane % NQ

    nc.compile()
    return nc


# ------------------------------------------------------------------- driver
def kernel(X, vertex, edges, X0, n_edges, w_b, w_a, w_c, b_b, b_a, b_c):
    from concourse.bass_utils import run_bass_kernel_spmd
    import ml_dtypes

    bf = ml_dtypes.bfloat16
    X = np.ascontiguousarray(np.asarray(X, dtype=np.float32))
    vertex = np.asarray(vertex).astype(np.int64)
    edges = np.asarray(edges).astype(np.int64)
    w_b = np.asarray(w_b, dtype=np.float32)
    w_a = np.asarray(w_a, dtype=np.float32)
    w_c = np.asarray(w_c, dtype=np.float32)
    b_b = np.asarray(b_b, dtype=np.float32).reshape(1, F)
    b_a = np.asarray(b_a, dtype=np.float32).reshape(1, F)
    b_c = np.asarray(b_c, dtype=np.float32).reshape(1, F)

    meta = _route(vertex, edges)
    if meta is None:
        return _numpy_fallback(X, vertex, edges, w_b, w_a, w_c, b_b, b_a, b_c)

    if _STATE.get("sig") != meta["sig"]:
        _STATE["nc"] = _build_program(meta)
        _STATE["sig"] = meta["sig"]
    nc = _STATE["nc"]
    in_maps = _make_in_maps(meta, X, vertex, w_b, w_a, w_c, b_b, b_a, b_c)

    res = run_bass_kernel_spmd(nc, in_maps, list(range(NC)))
    center = np.empty((N, F), np.float32)
    hl = np.empty((N, F), np.float32)
    hr = np.empty((N, F), np.float32)
    for m in range(NC):
        o = np.asarray(res.results[m]["out3T"]).astype(np.float32)
        sl = slice(m * NODE_SH, (m + 1) * NODE_SH)
        center[sl] = o[:, :NODE_SH].T
        hl[sl] = o[:, NODE_SH_P:NODE_SH_P + NODE_SH].T
        hr[sl] = o[:, 2 * NODE_SH_P:2 * NODE_SH_P + NODE_SH].T
    return center, hl, hr


def _make_in_maps(meta, X, vertex, w_b, w_a, w_c, b_b, b_a, b_c):
    import ml_dtypes
    bf = ml_dtypes.bfloat16
    Xbf = X.astype(bf)
    deg = np.bincount(vertex, minlength=N).astype(np.float32)
    XD = (X * deg[:, None]).astype(np.float32)

    soffA, blkA = meta["soffA"], meta["blkA"]
    TOTA = int(soffA[-1])
    maxblkA = int(max(soffA[min(4 * w + 4, NSUBA)] - soffA[4 * w]
                      for w in range(AW)))
    emax = max(maxblkA, meta["emaxC"])
    iota_np = np.ascontiguousarray(
        np.tile(np.arange(128, dtype=np.float16), (128, emax)))
    ident_np = np.ascontiguousarray(np.eye(128, dtype=np.float32).astype(bf))
    wcasts = {nm: np.ascontiguousarray(w.astype(bf))
              for nm, w in (("wb1", w_b[:F]), ("wb2", w_b[F:]),
                            ("wa1", w_a[:F]), ("wa2", w_a[F:]),
                            ("wc1", w_c[:F]), ("wc2", w_c[F:]))}
    bcols_np = np.ascontiguousarray(
        np.stack([b_b.ravel(), b_a.ravel(), b_c.ravel()], axis=1)
        .astype(np.float32))

    in_maps = []
    for m in range(NC):
        # host-gathered stage A token stream in the [128, block, F] layout
        xva = (Xbf[meta["gaA"][m]].reshape(TOTA, 128, F)
               .transpose(1, 0, 2))
        xdm = np.zeros((128, NODE_SH_P), np.float32)
        xdm[:, :NODE_SH] = XD[m * NODE_SH:(m + 1) * NODE_SH].T
        im = {
            "xva": np.ascontiguousarray(xva.reshape(128, TOTA * F)),
            "gaC": _wrap16v([meta["gaC"][m, k, :int(meta["cntC"][k])]
                             for k in range(2 * NSC)]),
            "sgA": np.ascontiguousarray(
                meta["sgA"][m].reshape(TOTA, 128).T.astype(np.float16)),
            "sgC": _segmb_img(meta["sgC"][m], meta["entC"], meta["emaxC"]),
            "xdt": np.ascontiguousarray(xdm.astype(bf)),
            "axdt": np.ascontiguousarray(np.abs(xdm).astype(bf)),
            "bcols": bcols_np,
            "iota": iota_np, "ident": ident_np,
        }
        im.update(wcasts)
        in_maps.append(im)
    return in_maps
